# revision 7
# baseline (speedup 1.0000x reference)
"""DecoderRNN single-step decode on 8 Trainium2 NeuronCores.

Strategy (tensor-parallel, everything sharded):
  - Host gathers the embedding row (pure indexing) and pre-transposes /
    shards all weights per core.
  - Core k computes h_new[k*128:(k+1)*128] (GRU slices), its 256 rows of
    attention scores, a partial ctx over its encoder shard, its 128-slice
    of comb, and its ~6283-row shard of W_out logits.
  - Cross-core exchanges use 5 small collectives: AllGather(h_new),
    AllGather(scores), AllReduce(ctx), AllGather(comb),
    AllGather(log-softmax stats).
  - log_softmax: per-core max m_k / sum s_k = sum exp(l - m_k); global
    C = M + log(sum_k s_k * exp(m_k - M)); each core outputs l - C.

Self-contained: shapes hardcoded, no sibling imports.
"""

import os
import numpy as np

import concourse.bacc as bacc
import concourse.bass as bass
import concourse.tile as tile
import concourse.mybir as mybir
from concourse.bass_utils import run_bass_kernel_spmd

F32 = mybir.dt.float32
AX = mybir.AxisListType.X
ALU = mybir.AluOpType
ACT = mybir.ActivationFunctionType

N_CORES = 8
H = 1024
V = 50257
S = 2048
HC = H // 128          # 8 h-chunks
SPC = S // N_CORES     # 256 encoder rows per core
VPC = 6400             # padded W_out rows per core (50 tiles of 128)
NVT = VPC // 128       # 50 v-tiles per core
BW = 640               # W_out DMA block width (5 v-tiles)
NB = VPC // BW         # 10 v-blocks
NEG = -1.0e9           # pad bias so padded logits never matter

_ROWS = [6283] * 7 + [V - 7 * 6283]   # real W_out rows per core
_OFFS = np.cumsum([0] + _ROWS)

_NC_CACHE = {}


def _build_nc():
    if "nc" in _NC_CACHE:
        return _NC_CACHE["nc"]
    nc = bacc.Bacc("TRN2", target_bir_lowering=False, debug=False,
                   num_devices=N_CORES)
    rg = [list(range(N_CORES))]

    # ---- per-core inputs --------------------------------------------------
    x8_d = nc.dram_tensor("x8", [128, HC], F32, kind="ExternalInput")
    h8_d = nc.dram_tensor("h8", [128, HC], F32, kind="ExternalInput")
    hown_d = nc.dram_tensor("hown", [128, 1], F32, kind="ExternalInput")
    wihT_d = nc.dram_tensor("wihT", [H, 384], F32, kind="ExternalInput")
    whhT_d = nc.dram_tensor("whhT", [H, 384], F32, kind="ExternalInput")
    brz_d = nc.dram_tensor("brz", [128, 2], F32, kind="ExternalInput")
    bin_d = nc.dram_tensor("bin", [128, 1], F32, kind="ExternalInput")
    bhn_d = nc.dram_tensor("bhn", [128, 1], F32, kind="ExternalInput")
    encT_d = nc.dram_tensor("encT", [H, SPC], F32, kind="ExternalInput")
    encN_d = nc.dram_tensor("encN", [SPC, H], F32, kind="ExternalInput")
    wcbT_d = nc.dram_tensor("wcbT", [2 * H, 128], F32, kind="ExternalInput")
    bcb_d = nc.dram_tensor("bcb", [128, 1], F32, kind="ExternalInput")
    woutT_d = nc.dram_tensor("woutT", [H, VPC], F32, kind="ExternalInput")
    bout_d = nc.dram_tensor("bout", [1, VPC], F32, kind="ExternalInput")
    ident_d = nc.dram_tensor("ident", [128, 128], F32, kind="ExternalInput")
    ones_d = nc.dram_tensor("ones", [128, 1], F32, kind="ExternalInput")
    onesr_d = nc.dram_tensor("onesr", [1, 128], F32, kind="ExternalInput")

    # ---- outputs ----------------------------------------------------------
    out_lg_d = nc.dram_tensor("out_logits", [128, NVT], F32,
                              kind="ExternalOutput")
    out_hn_d = nc.dram_tensor("out_hnew", [H], F32, kind="ExternalOutput")
    out_at_d = nc.dram_tensor("out_attn", [S], F32, kind="ExternalOutput")

    with tile.TileContext(nc) as tc:
        with (
            tc.tile_pool(name="w", bufs=1) as w,          # persistent weights
            tc.tile_pool(name="wo", bufs=16) as wo,       # W_out stream
            tc.tile_pool(name="sb", bufs=1) as sb,        # small working tiles
            tc.tile_pool(name="psA", bufs=2, space="PSUM") as psA,
            tc.tile_pool(name="psB", bufs=2, space="PSUM") as psB,
            tc.tile_pool(name="psW", bufs=3, space="PSUM") as psW,
            tc.tile_pool(name="psT", bufs=1, space="PSUM") as psT,
            tc.tile_pool(name="dr", bufs=1, space="DRAM") as dr,
            tc.tile_pool(name="drs", bufs=1, space="DRAM") as drs,
        ):
            # ---- front-end weight / vector loads (priority first) --------
            ident = w.tile([128, 128], F32)
            nc.sync.dma_start(out=ident[:], in_=ident_d[:])
            ones = w.tile([128, 1], F32)
            nc.sync.dma_start(out=ones[:], in_=ones_d[:])
            onesr = w.tile([1, 128], F32)
            nc.sync.dma_start(out=onesr[:], in_=onesr_d[:])
            x8 = w.tile([128, HC], F32)
            nc.sync.dma_start(out=x8[:], in_=x8_d[:])
            h8 = w.tile([128, HC], F32)
            nc.sync.dma_start(out=h8[:], in_=h8_d[:])
            hown = w.tile([128, 1], F32)
            nc.sync.dma_start(out=hown[:], in_=hown_d[:])
            brz = w.tile([128, 2], F32)
            nc.sync.dma_start(out=brz[:], in_=brz_d[:])
            bin_ = w.tile([128, 1], F32)
            nc.sync.dma_start(out=bin_[:], in_=bin_d[:])
            bhn = w.tile([128, 1], F32)
            nc.sync.dma_start(out=bhn[:], in_=bhn_d[:])
            bcb = w.tile([128, 1], F32)
            nc.sync.dma_start(out=bcb[:], in_=bcb_d[:])

            wih_sb = []
            whh_sb = []
            for c in range(HC):
                t1 = w.tile([128, 384], F32, name=f"wih_{c}")
                nc.sync.dma_start(out=t1[:],
                                  in_=wihT_d[c * 128:(c + 1) * 128, :])
                wih_sb.append(t1)
                t2 = w.tile([128, 384], F32, name=f"whh_{c}")
                nc.sync.dma_start(out=t2[:],
                                  in_=whhT_d[c * 128:(c + 1) * 128, :])
                whh_sb.append(t2)
            encT_sb = []
            for c in range(HC):
                t = w.tile([128, SPC], F32, name=f"encT_{c}")
                nc.sync.dma_start(out=t[:],
                                  in_=encT_d[c * 128:(c + 1) * 128, :])
                encT_sb.append(t)
            encN_sb = []
            for t_i in range(2):
                t = w.tile([128, H], F32, name=f"encN_{t_i}")
                nc.sync.dma_start(out=t[:],
                                  in_=encN_d[t_i * 128:(t_i + 1) * 128, :])
                encN_sb.append(t)
            wcb_sb = []
            for c in range(16):
                t = w.tile([128, 128], F32, name=f"wcb_{c}")
                nc.sync.dma_start(out=t[:],
                                  in_=wcbT_d[c * 128:(c + 1) * 128, :])
                wcb_sb.append(t)
            bout_sb = w.tile([1, VPC], F32)
            nc.sync.dma_start(out=bout_sb[:], in_=bout_d[:])

            # ---- W_out stream DMAs (fill remaining bandwidth) -------------
            wo_tiles = [[None] * HC for _ in range(NB)]
            for vb in range(NB):
                for c in range(HC):
                    t = wo.tile([128, BW], F32, tag="wo", name=f"wo_{vb}_{c}")
                    nc.sync.dma_start(
                        out=t[:],
                        in_=woutT_d[c * 128:(c + 1) * 128,
                                    vb * BW:(vb + 1) * BW])
                    wo_tiles[vb][c] = t

            # ---- P1: GRU gates -------------------------------------------
            xr = sb.tile([128, HC], F32)
            nc.scalar.activation(xr[:], x8[:], ACT.Relu)
            gi = psA.tile([128, 3], F32, tag="gru")
            gh = psA.tile([128, 3], F32, tag="gru")
            for g in range(3):
                for c in range(HC):
                    nc.tensor.matmul(gi[:, g:g + 1],
                                     lhsT=wih_sb[c][:, g * 128:(g + 1) * 128],
                                     rhs=xr[:, c:c + 1],
                                     start=(c == 0), stop=(c == HC - 1))
            for g in range(3):
                for c in range(HC):
                    nc.tensor.matmul(gh[:, g:g + 1],
                                     lhsT=whh_sb[c][:, g * 128:(g + 1) * 128],
                                     rhs=h8[:, c:c + 1],
                                     start=(c == 0), stop=(c == HC - 1))
            gisb = sb.tile([128, 3], F32)
            nc.vector.tensor_copy(gisb[:], gi[:])
            rzp = sb.tile([128, 2], F32)
            nc.vector.tensor_add(rzp[:], gisb[:, 0:2], gh[:, 0:2])
            nc.vector.tensor_add(rzp[:], rzp[:], brz[:])
            rz = sb.tile([128, 2], F32)
            nc.scalar.activation(rz[:], rzp[:], ACT.Sigmoid)
            npre = sb.tile([128, 1], F32)
            nc.vector.tensor_add(npre[:], gisb[:, 2:3], bin_[:])
            hnp = sb.tile([128, 1], F32)
            nc.vector.tensor_add(hnp[:], gh[:, 2:3], bhn[:])
            rhn = sb.tile([128, 1], F32)
            nc.vector.tensor_mul(rhn[:], rz[:, 0:1], hnp[:])
            nc.vector.tensor_add(npre[:], npre[:], rhn[:])
            n_t = sb.tile([128, 1], F32)
            nc.scalar.activation(n_t[:], npre[:], ACT.Tanh)
            # h_new = n + z*(h - n)
            dmn = sb.tile([128, 1], F32)
            nc.vector.tensor_sub(dmn[:], hown[:], n_t[:])
            nc.vector.tensor_mul(dmn[:], rz[:, 1:2], dmn[:])
            hno = sb.tile([128, 1], F32)
            nc.vector.tensor_add(hno[:], n_t[:], dmn[:])

            # ---- P2: AllGather h_new -------------------------------------
            hn_b = dr.tile([128], F32)
            nc.sync.dma_start(out=hn_b[:].rearrange("(p f) -> p f", f=1),
                              in_=hno[:])
            hn_g = drs.tile([H], F32, addr_space="Shared")
            nc.gpsimd.collective_compute("AllGather", ALU.bypass,
                                         replica_groups=rg,
                                         ins=[hn_b[:]], outs=[hn_g[:]])
            nc.sync.dma_start(out=out_hn_d[:], in_=hn_g[:])
            hnew8 = sb.tile([128, HC], F32)
            nc.sync.dma_start(out=hnew8[:],
                              in_=hn_g[:].rearrange("(f p) -> p f", p=128))

            # ---- P3: attention scores ------------------------------------
            sc = psB.tile([128, 2], F32, tag="att")
            for t_i in range(2):
                for c in range(HC):
                    nc.tensor.matmul(
                        sc[:, t_i:t_i + 1],
                        lhsT=encT_sb[c][:, t_i * 128:(t_i + 1) * 128],
                        rhs=hnew8[:, c:c + 1],
                        start=(c == 0), stop=(c == HC - 1))
            sc_sb = sb.tile([128, 2], F32)
            nc.vector.tensor_copy(sc_sb[:], sc[:])
            sc_b = dr.tile([SPC], F32)
            nc.sync.dma_start(out=sc_b[:].rearrange("(f p) -> p f", p=128),
                              in_=sc_sb[:])
            sc_g = drs.tile([S], F32, addr_space="Shared")
            nc.gpsimd.collective_compute("AllGather", ALU.bypass,
                                         replica_groups=rg,
                                         ins=[sc_b[:]], outs=[sc_g[:]])

            # ---- P4: softmax ---------------------------------------------
            scf = sb.tile([128, 16], F32)
            nc.sync.dma_start(out=scf[:],
                              in_=sc_g[:].rearrange("(p f) -> p f", f=16))
            m1 = sb.tile([128, 1], F32)
            nc.vector.tensor_reduce(m1[:], scf[:], axis=AX, op=ALU.max)
            m1t = psT.tile([1, 128], F32, tag="tp", name="m1t")
            nc.tensor.transpose(m1t[:], m1[:], ident[:])
            mx = sb.tile([1, 1], F32)
            nc.vector.tensor_reduce(mx[:], m1t[:], axis=AX, op=ALU.max)
            nmx = sb.tile([1, 1], F32)
            nc.scalar.mul(nmx[:], mx[:], -1.0)
            nmb_p = psT.tile([128, 1], F32, tag="tp", name="nmb_p")
            nc.tensor.matmul(nmb_p[:], lhsT=onesr[:], rhs=nmx[:],
                             start=True, stop=True)
            nmb = sb.tile([128, 1], F32)
            nc.vector.tensor_copy(nmb[:], nmb_p[:])
            ex = sb.tile([128, 16], F32)
            sums = sb.tile([128, 1], F32)
            nc.scalar.activation(ex[:], scf[:], ACT.Exp, bias=nmb[:],
                                 accum_out=sums[:])
            z_p = psT.tile([1, 1], F32, tag="tp", name="z_p")
            nc.tensor.matmul(z_p[:], lhsT=sums[:], rhs=ones[:],
                             start=True, stop=True)
            invz = sb.tile([1, 1], F32)
            nc.vector.reciprocal(invz[:], z_p[:])
            izb_p = psT.tile([128, 1], F32, tag="tp", name="izb_p")
            nc.tensor.matmul(izb_p[:], lhsT=onesr[:], rhs=invz[:],
                             start=True, stop=True)
            izb = sb.tile([128, 1], F32)
            nc.vector.tensor_copy(izb[:], izb_p[:])
            attn = sb.tile([128, 16], F32)
            nc.vector.tensor_scalar_mul(attn[:], ex[:], izb[:])
            nc.sync.dma_start(out=out_at_d[:].rearrange("(p f) -> p f", f=16),
                              in_=attn[:])
            # local attention weights (own 256 scores live in sc psum)
            el = sb.tile([128, 2], F32)
            nc.scalar.activation(el[:], sc_sb[:], ACT.Exp, bias=nmb[:])
            al = sb.tile([128, 2], F32)
            nc.vector.tensor_scalar_mul(al[:], el[:], izb[:])

            # ---- P5: partial ctx over own encoder shard ------------------
            ctxp = psB.tile([128, HC], F32, tag="att")
            for j in range(HC):
                for t_i in range(2):
                    nc.tensor.matmul(
                        ctxp[:, j:j + 1],
                        lhsT=encN_sb[t_i][:, j * 128:(j + 1) * 128],
                        rhs=al[:, t_i:t_i + 1],
                        start=(t_i == 0), stop=(t_i == 1))
            ctx_sb = sb.tile([128, HC], F32)
            nc.vector.tensor_copy(ctx_sb[:], ctxp[:])
            ctx_b = dr.tile([H], F32)
            nc.sync.dma_start(out=ctx_b[:].rearrange("(p f) -> p f", f=HC),
                              in_=ctx_sb[:])
            ctx_g = drs.tile([H], F32, addr_space="Shared", name="ctx_g")
            nc.gpsimd.collective_compute("AllReduce", ALU.add,
                                         replica_groups=rg,
                                         ins=[ctx_b[:]], outs=[ctx_g[:]])
            ctx8 = sb.tile([128, HC], F32)
            nc.sync.dma_start(out=ctx8[:],
                              in_=ctx_g[:].rearrange("(p f) -> p f", f=HC))

            # ---- P6: comb slice ------------------------------------------
            cbp = psB.tile([128, 1], F32, tag="att")
            for c in range(16):
                rhs = ctx8[:, c:c + 1] if c < HC else hnew8[:, c - HC:c - HC + 1]
                nc.tensor.matmul(cbp[:], lhsT=wcb_sb[c][:], rhs=rhs,
                                 start=(c == 0), stop=(c == 15))
            cb = sb.tile([128, 1], F32)
            nc.scalar.activation(cb[:], cbp[:], ACT.Tanh, bias=bcb[:])
            cb_b = dr.tile([128], F32)
            nc.sync.dma_start(out=cb_b[:].rearrange("(p f) -> p f", f=1),
                              in_=cb[:])
            cb_g = drs.tile([H], F32, addr_space="Shared", name="cb_g")
            nc.gpsimd.collective_compute("AllGather", ALU.bypass,
                                         replica_groups=rg,
                                         ins=[cb_b[:]], outs=[cb_g[:]])
            comb8 = sb.tile([128, HC], F32)
            nc.sync.dma_start(out=comb8[:],
                              in_=cb_g[:].rearrange("(f p) -> p f", p=128))

            # ---- P7: W_out matvec + streaming stats ----------------------
            logits = sb.tile([128, NVT], F32)
            rmax = sb.tile([128, 1], F32)
            nc.vector.memset(rmax[:], NEG)
            for vb in range(NB):
                ps = psW.tile([128, BW // 128], F32, tag="wops",
                              name=f"wops_{vb}")
                for vt in range(BW // 128):
                    v_i = vb * (BW // 128) + vt
                    for c in range(HC):
                        nc.tensor.matmul(
                            ps[:, vt:vt + 1],
                            lhsT=wo_tiles[vb][c][:, vt * 128:(vt + 1) * 128],
                            rhs=comb8[:, c:c + 1],
                            start=(c == 0), stop=False)
                    nc.tensor.matmul(
                        ps[:, vt:vt + 1],
                        lhsT=bout_sb[0:1, v_i * 128:(v_i + 1) * 128],
                        rhs=ones[0:1, 0:1],
                        start=False, stop=True)
                nvb = BW // 128
                nc.scalar.copy(logits[:, vb * nvb:(vb + 1) * nvb], ps[:])
                bm = sb.tile([128, 1], F32, tag="bm", name=f"bm_{vb}")
                nc.vector.tensor_reduce(bm[:], ps[:], axis=AX, op=ALU.max)
                nc.vector.tensor_max(rmax[:], rmax[:], bm[:])

            # ---- P8: log-softmax stats + final ---------------------------
            rmt = psT.tile([1, 128], F32, tag="tp", name="rmt")
            nc.tensor.transpose(rmt[:], rmax[:], ident[:])
            mk = sb.tile([1, 1], F32)
            nc.vector.tensor_reduce(mk[:], rmt[:], axis=AX, op=ALU.max)
            nmk = sb.tile([1, 1], F32)
            nc.scalar.mul(nmk[:], mk[:], -1.0)
            nmkb_p = psT.tile([128, 1], F32, tag="tp", name="nmkb_p")
            nc.tensor.matmul(nmkb_p[:], lhsT=onesr[:], rhs=nmk[:],
                             start=True, stop=True)
            nmkb = sb.tile([128, 1], F32)
            nc.vector.tensor_copy(nmkb[:], nmkb_p[:])
            esc = sb.tile([128, NVT], F32)
            sumk = sb.tile([128, 1], F32)
            nc.scalar.activation(esc[:], logits[:], ACT.Exp, bias=nmkb[:],
                                 accum_out=sumk[:])
            sk_p = psT.tile([1, 1], F32, tag="tp", name="sk_p")
            nc.tensor.matmul(sk_p[:], lhsT=sumk[:], rhs=ones[:],
                             start=True, stop=True)
            st2 = sb.tile([1, 2], F32)
            nc.vector.tensor_copy(st2[0:1, 0:1], mk[:])
            nc.vector.tensor_copy(st2[0:1, 1:2], sk_p[:])
            st_b = dr.tile([2], F32)
            nc.sync.dma_start(out=st_b[:].rearrange("(p f) -> p f", p=1),
                              in_=st2[:])
            st_g = drs.tile([2 * N_CORES], F32, addr_space="Shared",
                            name="st_g")
            nc.gpsimd.collective_compute("AllGather", ALU.bypass,
                                         replica_groups=rg,
                                         ins=[st_b[:]], outs=[st_g[:]])
            sg = sb.tile([1, 16], F32)
            nc.sync.dma_start(out=sg[:],
                              in_=st_g[:].rearrange("(p f) -> p f", p=1))
            sgv = sg[:].rearrange("p (e two) -> p two e", two=2)
            mview = sgv[:, 0:1, :].rearrange("p a e -> p (a e)")
            sview = sgv[:, 1:2, :].rearrange("p a e -> p (a e)")
            gm = sb.tile([1, 1], F32)
            nc.vector.tensor_reduce(gm[:], mview, axis=AX, op=ALU.max)
            ngm = sb.tile([1, 1], F32)
            nc.scalar.mul(ngm[:], gm[:], -1.0)
            em8 = sb.tile([1, 8], F32)
            nc.scalar.activation(em8[:], mview, ACT.Exp, bias=ngm[:])
            zt8 = sb.tile([1, 8], F32)
            nc.vector.tensor_mul(zt8[:], em8[:], sview)
            zz = sb.tile([1, 1], F32)
            nc.vector.tensor_reduce(zz[:], zt8[:], axis=AX, op=ALU.add)
            lz = sb.tile([1, 1], F32)
            nc.scalar.activation(lz[:], zz[:], ACT.Ln)
            cc = sb.tile([1, 1], F32)
            nc.vector.tensor_add(cc[:], gm[:], lz[:])
            ncc = sb.tile([1, 1], F32)
            nc.scalar.mul(ncc[:], cc[:], -1.0)
            nccb_p = psT.tile([128, 1], F32, tag="tp", name="nccb_p")
            nc.tensor.matmul(nccb_p[:], lhsT=onesr[:], rhs=ncc[:],
                             start=True, stop=True)
            nccb = sb.tile([128, 1], F32)
            nc.vector.tensor_copy(nccb[:], nccb_p[:])
            outf = sb.tile([128, NVT], F32)
            nc.vector.tensor_scalar_add(outf[:], logits[:], nccb[:])
            nc.sync.dma_start(out=out_lg_d[:], in_=outf[:])

    nc.compile()
    _NC_CACHE["nc"] = nc
    return nc


def _prep_inputs(input_ids, hidden, encoder_outputs, emb_table, W_ih, W_hh,
                 b_ih, b_hh, W_comb, b_comb, W_out, b_out):
    """Shard + pre-layout all inputs per core (host-side, numpy)."""
    f = np.float32
    x_row = np.ascontiguousarray(emb_table[int(input_ids[0])], dtype=f)
    h_row = np.ascontiguousarray(hidden.reshape(H), dtype=f)
    x8 = np.ascontiguousarray(x_row.reshape(HC, 128).T)
    h8 = np.ascontiguousarray(h_row.reshape(HC, 128).T)
    ident = np.eye(128, dtype=f)
    ones = np.ones((128, 1), dtype=f)
    onesr = np.ones((1, 128), dtype=f)
    bsum = (b_ih + b_hh).astype(f)

    encT_full = np.ascontiguousarray(encoder_outputs.T, dtype=f)  # (H, S)
    W_ih = np.asarray(W_ih, dtype=f)
    W_hh = np.asarray(W_hh, dtype=f)
    W_comb = np.asarray(W_comb, dtype=f)
    W_out = np.asarray(W_out, dtype=f)
    b_out = np.asarray(b_out, dtype=f)

    in_maps = []
    for k in range(N_CORES):
        sl = slice(k * 128, (k + 1) * 128)
        # gate-sliced, transposed GRU weights: (H, 384) cols = [r|z|n]
        wihT = np.empty((H, 384), dtype=f)
        whhT = np.empty((H, 384), dtype=f)
        for g in range(3):
            wihT[:, g * 128:(g + 1) * 128] = W_ih[g * H + k * 128:
                                                  g * H + (k + 1) * 128, :].T
            whhT[:, g * 128:(g + 1) * 128] = W_hh[g * H + k * 128:
                                                  g * H + (k + 1) * 128, :].T
        brz = np.stack([bsum[0 * H + k * 128:0 * H + (k + 1) * 128],
                        bsum[1 * H + k * 128:1 * H + (k + 1) * 128]], axis=1)
        bin_ = np.asarray(b_ih[2 * H + k * 128:2 * H + (k + 1) * 128],
                          dtype=f).reshape(128, 1)
        bhn = np.asarray(b_hh[2 * H + k * 128:2 * H + (k + 1) * 128],
                         dtype=f).reshape(128, 1)
        encT = np.ascontiguousarray(encT_full[:, k * SPC:(k + 1) * SPC])
        encN = np.ascontiguousarray(
            encoder_outputs[k * SPC:(k + 1) * SPC, :], dtype=f)
        wcbT = np.ascontiguousarray(W_comb[sl, :].T)        # (2H, 128)
        bcb = np.asarray(b_comb[sl], dtype=f).reshape(128, 1)
        lo, hi = int(_OFFS[k]), int(_OFFS[k + 1])
        r = hi - lo
        woutT = np.zeros((H, VPC), dtype=f)
        woutT[:, :r] = W_out[lo:hi, :].T
        bout = np.full((1, VPC), NEG, dtype=f)
        bout[0, :r] = b_out[lo:hi]
        in_maps.append({
            "x8": x8, "h8": h8,
            "hown": np.ascontiguousarray(h_row[sl]).reshape(128, 1),
            "wihT": wihT, "whhT": whhT, "brz": np.ascontiguousarray(brz),
            "bin": bin_, "bhn": bhn, "encT": encT, "encN": encN,
            "wcbT": wcbT, "bcb": bcb, "woutT": woutT, "bout": bout,
            "ident": ident, "ones": ones, "onesr": onesr,
        })
    return in_maps


def _assemble(results):
    log_probs = np.empty((1, V), dtype=np.float32)
    for k in range(N_CORES):
        lg = np.asarray(results[k]["out_logits"]).reshape(128, NVT)
        shard = lg.T.reshape(VPC)
        lo, hi = int(_OFFS[k]), int(_OFFS[k + 1])
        log_probs[0, lo:hi] = shard[:hi - lo]
    h_new = np.asarray(results[0]["out_hnew"]).reshape(1, 1, H)
    attn = np.asarray(results[0]["out_attn"]).reshape(S)
    return log_probs, h_new, attn


_LAST_EXEC_NS = {"ns": None}


def kernel(**inputs):
    nc = _build_nc()
    in_maps = _prep_inputs(**inputs)
    if os.environ.get("KERNEL_SIM"):
        from concourse.bass_interp import MultiCoreSim
        sim = MultiCoreSim(nc, N_CORES)
        for i in range(N_CORES):
            for name, arr in in_maps[i].items():
                sim.cores[i].tensor(name)[:] = arr
        sim.simulate(check_with_hw=False)
        results = [{name: np.asarray(sim.cores[i].mem_tensor(name))
                    for name in ("out_logits", "out_hnew", "out_attn")}
                   for i in range(N_CORES)]
    else:
        trace = bool(os.environ.get("KERNEL_TRACE"))
        res = run_bass_kernel_spmd(nc, in_maps, list(range(N_CORES)),
                                   trace=trace)
        _LAST_EXEC_NS["ns"] = res.exec_time_ns
        results = res.results
    return _assemble(results)


# revision 10
# speedup vs baseline: 1.4025x; 1.4025x over previous
"""DecoderRNN single-step decode on 8 Trainium2 NeuronCores.

Strategy (tensor-parallel, everything sharded):
  - Host gathers the embedding row (pure indexing) and pre-transposes /
    shards all weights per core.
  - Core k computes h_new[k*128:(k+1)*128] (GRU slices), its 256 rows of
    attention scores, a partial ctx over its encoder shard, its 128-slice
    of comb, and its ~6283-row shard of W_out logits.
  - Cross-core exchanges use 5 small collectives: AllGather(h_new),
    AllGather(scores), AllReduce(ctx), AllGather(comb),
    AllGather(log-softmax stats).
  - log_softmax: per-core max m_k / sum s_k = sum exp(l - m_k); global
    C = M + log(sum_k s_k * exp(m_k - M)); each core outputs l - C.

Self-contained: shapes hardcoded, no sibling imports.
"""

import os
import numpy as np

import concourse.bacc as bacc
import concourse.bass as bass
import concourse.tile as tile
import concourse.mybir as mybir
from concourse.bass_utils import run_bass_kernel_spmd

F32 = mybir.dt.float32
BF16 = mybir.dt.bfloat16
AX = mybir.AxisListType.X
ALU = mybir.AluOpType
ACT = mybir.ActivationFunctionType

N_CORES = 8
H = 1024
V = 50257
S = 2048
HC = H // 128          # 8 h-chunks
SPC = S // N_CORES     # 256 encoder rows per core
VPC = 6400             # padded W_out rows per core (50 tiles of 128)
NVT = VPC // 128       # 50 v-tiles per core
BW = 640               # W_out DMA block width (5 v-tiles)
NB = VPC // BW         # 10 v-blocks
NEG = -1.0e9           # pad bias so padded logits never matter

_ROWS = [6283] * 7 + [V - 7 * 6283]   # real W_out rows per core
_OFFS = np.cumsum([0] + _ROWS)

_NC_CACHE = {}


def _build_nc():
    if "nc" in _NC_CACHE:
        return _NC_CACHE["nc"]
    nc = bacc.Bacc("TRN2", target_bir_lowering=False, debug=False,
                   num_devices=N_CORES)
    rg = [list(range(N_CORES))]

    # ---- per-core inputs --------------------------------------------------
    x8_d = nc.dram_tensor("x8", [128, HC], F32, kind="ExternalInput")
    h8_d = nc.dram_tensor("h8", [128, HC], F32, kind="ExternalInput")
    hown_d = nc.dram_tensor("hown", [128, 1], F32, kind="ExternalInput")
    wihT_d = nc.dram_tensor("wihT", [H, 384], F32, kind="ExternalInput")
    whhT_d = nc.dram_tensor("whhT", [H, 384], F32, kind="ExternalInput")
    brz_d = nc.dram_tensor("brz", [128, 2], F32, kind="ExternalInput")
    bin_d = nc.dram_tensor("bin", [128, 1], F32, kind="ExternalInput")
    bhn_d = nc.dram_tensor("bhn", [128, 1], F32, kind="ExternalInput")
    encT_d = nc.dram_tensor("encT", [H, SPC], F32, kind="ExternalInput")
    encN_d = nc.dram_tensor("encN", [SPC, H], F32, kind="ExternalInput")
    wcbT_d = nc.dram_tensor("wcbT", [2 * H, 128], F32, kind="ExternalInput")
    bcb_d = nc.dram_tensor("bcb", [128, 1], F32, kind="ExternalInput")
    woutT_d = nc.dram_tensor("woutT", [H, VPC], BF16, kind="ExternalInput")
    bout_d = nc.dram_tensor("bout2d", [128, NVT], F32, kind="ExternalInput")
    ident_d = nc.dram_tensor("ident", [128, 128], F32, kind="ExternalInput")
    ones_d = nc.dram_tensor("ones", [128, 1], F32, kind="ExternalInput")
    onesr_d = nc.dram_tensor("onesr", [1, 128], F32, kind="ExternalInput")

    # ---- outputs ----------------------------------------------------------
    out_lg_d = nc.dram_tensor("out_logits", [128, NVT], F32,
                              kind="ExternalOutput")
    out_hn_d = nc.dram_tensor("out_hnew", [H], F32, kind="ExternalOutput")
    out_at_d = nc.dram_tensor("out_attn", [S], F32, kind="ExternalOutput")

    with tile.TileContext(nc) as tc:
        with (
            tc.tile_pool(name="w", bufs=1) as w,          # persistent weights
            tc.tile_pool(name="wo", bufs=16) as wo,       # W_out stream
            tc.tile_pool(name="sb", bufs=1) as sb,        # small working tiles
            tc.tile_pool(name="psA", bufs=2, space="PSUM") as psA,
            tc.tile_pool(name="psB", bufs=2, space="PSUM") as psB,
            tc.tile_pool(name="psW", bufs=3, space="PSUM") as psW,
            tc.tile_pool(name="psT", bufs=1, space="PSUM") as psT,
            tc.tile_pool(name="dr", bufs=1, space="DRAM") as dr,
            tc.tile_pool(name="drs", bufs=1, space="DRAM") as drs,
        ):
            # ---- front-end weight / vector loads (priority first) --------
            ident = w.tile([128, 128], F32)
            nc.sync.dma_start(out=ident[:], in_=ident_d[:])
            ones = w.tile([128, 1], F32)
            nc.sync.dma_start(out=ones[:], in_=ones_d[:])
            onesr = w.tile([1, 128], F32)
            nc.sync.dma_start(out=onesr[:], in_=onesr_d[:])
            x8 = w.tile([128, HC], F32)
            nc.sync.dma_start(out=x8[:], in_=x8_d[:])
            h8 = w.tile([128, HC], F32)
            nc.sync.dma_start(out=h8[:], in_=h8_d[:])
            hown = w.tile([128, 1], F32)
            nc.sync.dma_start(out=hown[:], in_=hown_d[:])
            brz = w.tile([128, 2], F32)
            nc.sync.dma_start(out=brz[:], in_=brz_d[:])
            bin_ = w.tile([128, 1], F32)
            nc.sync.dma_start(out=bin_[:], in_=bin_d[:])
            bhn = w.tile([128, 1], F32)
            nc.sync.dma_start(out=bhn[:], in_=bhn_d[:])
            bcb = w.tile([128, 1], F32)
            nc.sync.dma_start(out=bcb[:], in_=bcb_d[:])

            wih_sb = []
            whh_sb = []
            for c in range(HC):
                t1 = w.tile([128, 384], F32, name=f"wih_{c}")
                nc.sync.dma_start(out=t1[:],
                                  in_=wihT_d[c * 128:(c + 1) * 128, :])
                wih_sb.append(t1)
                t2 = w.tile([128, 384], F32, name=f"whh_{c}")
                nc.sync.dma_start(out=t2[:],
                                  in_=whhT_d[c * 128:(c + 1) * 128, :])
                whh_sb.append(t2)
            encT_sb = []
            for c in range(HC):
                t = w.tile([128, SPC], F32, name=f"encT_{c}")
                nc.sync.dma_start(out=t[:],
                                  in_=encT_d[c * 128:(c + 1) * 128, :])
                encT_sb.append(t)
            encN_sb = []
            for t_i in range(2):
                t = w.tile([128, H], F32, name=f"encN_{t_i}")
                nc.sync.dma_start(out=t[:],
                                  in_=encN_d[t_i * 128:(t_i + 1) * 128, :])
                encN_sb.append(t)
            wcb_sb = []
            for c in range(16):
                t = w.tile([128, 128], F32, name=f"wcb_{c}")
                nc.sync.dma_start(out=t[:],
                                  in_=wcbT_d[c * 128:(c + 1) * 128, :])
                wcb_sb.append(t)
            bout_sb = w.tile([128, NVT], F32)
            nc.sync.dma_start(out=bout_sb[:], in_=bout_d[:])

            # ---- W_out stream DMAs (fill remaining bandwidth) -------------
            wo_tiles = [[None] * HC for _ in range(NB)]
            for vb in range(NB):
                for c in range(HC):
                    t = wo.tile([128, BW], BF16, tag="wo", name=f"wo_{vb}_{c}")
                    nc.sync.dma_start(
                        out=t[:],
                        in_=woutT_d[c * 128:(c + 1) * 128,
                                    vb * BW:(vb + 1) * BW])
                    wo_tiles[vb][c] = t

            # ---- P1: GRU gates -------------------------------------------
            xr = sb.tile([128, HC], F32)
            nc.scalar.activation(xr[:], x8[:], ACT.Relu)
            gi = psA.tile([128, 3], F32, tag="gru")
            gh = psA.tile([128, 3], F32, tag="gru")
            for g in range(3):
                for c in range(HC):
                    nc.tensor.matmul(gi[:, g:g + 1],
                                     lhsT=wih_sb[c][:, g * 128:(g + 1) * 128],
                                     rhs=xr[:, c:c + 1],
                                     start=(c == 0), stop=(c == HC - 1))
            for g in range(3):
                for c in range(HC):
                    nc.tensor.matmul(gh[:, g:g + 1],
                                     lhsT=whh_sb[c][:, g * 128:(g + 1) * 128],
                                     rhs=h8[:, c:c + 1],
                                     start=(c == 0), stop=(c == HC - 1))
            gisb = sb.tile([128, 3], F32)
            nc.vector.tensor_copy(gisb[:], gi[:])
            rzp = sb.tile([128, 2], F32)
            nc.vector.tensor_add(rzp[:], gisb[:, 0:2], gh[:, 0:2])
            nc.vector.tensor_add(rzp[:], rzp[:], brz[:])
            rz = sb.tile([128, 2], F32)
            nc.scalar.activation(rz[:], rzp[:], ACT.Sigmoid)
            npre = sb.tile([128, 1], F32)
            nc.vector.tensor_add(npre[:], gisb[:, 2:3], bin_[:])
            hnp = sb.tile([128, 1], F32)
            nc.vector.tensor_add(hnp[:], gh[:, 2:3], bhn[:])
            rhn = sb.tile([128, 1], F32)
            nc.vector.tensor_mul(rhn[:], rz[:, 0:1], hnp[:])
            nc.vector.tensor_add(npre[:], npre[:], rhn[:])
            n_t = sb.tile([128, 1], F32)
            nc.scalar.activation(n_t[:], npre[:], ACT.Tanh)
            # h_new = n + z*(h - n)
            dmn = sb.tile([128, 1], F32)
            nc.vector.tensor_sub(dmn[:], hown[:], n_t[:])
            nc.vector.tensor_mul(dmn[:], rz[:, 1:2], dmn[:])
            hno = sb.tile([128, 1], F32)
            nc.vector.tensor_add(hno[:], n_t[:], dmn[:])

            # ---- P2: AllGather h_new -------------------------------------
            hn_b = dr.tile([128], F32)
            nc.sync.dma_start(out=hn_b[:].rearrange("(p f) -> p f", f=1),
                              in_=hno[:])
            hn_g = drs.tile([H], F32, addr_space="Shared")
            nc.gpsimd.collective_compute("AllGather", ALU.bypass,
                                         replica_groups=rg,
                                         ins=[hn_b[:]], outs=[hn_g[:]])
            nc.sync.dma_start(out=out_hn_d[:], in_=hn_g[:])
            hnew8 = sb.tile([128, HC], F32)
            nc.sync.dma_start(out=hnew8[:],
                              in_=hn_g[:].rearrange("(f p) -> p f", p=128))

            # ---- P3: attention scores ------------------------------------
            sc = psB.tile([128, 2], F32, tag="att")
            for t_i in range(2):
                for c in range(HC):
                    nc.tensor.matmul(
                        sc[:, t_i:t_i + 1],
                        lhsT=encT_sb[c][:, t_i * 128:(t_i + 1) * 128],
                        rhs=hnew8[:, c:c + 1],
                        start=(c == 0), stop=(c == HC - 1))
            sc_sb = sb.tile([128, 2], F32)
            nc.vector.tensor_copy(sc_sb[:], sc[:])
            sc_b = dr.tile([SPC], F32)
            nc.sync.dma_start(out=sc_b[:].rearrange("(f p) -> p f", p=128),
                              in_=sc_sb[:])
            sc_g = drs.tile([S], F32, addr_space="Shared")
            nc.gpsimd.collective_compute("AllGather", ALU.bypass,
                                         replica_groups=rg,
                                         ins=[sc_b[:]], outs=[sc_g[:]])

            # ---- P4: softmax ---------------------------------------------
            scf = sb.tile([128, 16], F32)
            nc.sync.dma_start(out=scf[:],
                              in_=sc_g[:].rearrange("(p f) -> p f", f=16))
            m1 = sb.tile([128, 1], F32)
            nc.vector.tensor_reduce(m1[:], scf[:], axis=AX, op=ALU.max)
            m1t = psT.tile([1, 128], F32, tag="tp", name="m1t")
            nc.tensor.transpose(m1t[:], m1[:], ident[:])
            mx = sb.tile([1, 1], F32)
            nc.vector.tensor_reduce(mx[:], m1t[:], axis=AX, op=ALU.max)
            nmx = sb.tile([1, 1], F32)
            nc.scalar.mul(nmx[:], mx[:], -1.0)
            nmb_p = psT.tile([128, 1], F32, tag="tp", name="nmb_p")
            nc.tensor.matmul(nmb_p[:], lhsT=onesr[:], rhs=nmx[:],
                             start=True, stop=True)
            nmb = sb.tile([128, 1], F32)
            nc.vector.tensor_copy(nmb[:], nmb_p[:])
            ex = sb.tile([128, 16], F32)
            sums = sb.tile([128, 1], F32)
            nc.scalar.activation(ex[:], scf[:], ACT.Exp, bias=nmb[:],
                                 accum_out=sums[:])
            z_p = psT.tile([1, 1], F32, tag="tp", name="z_p")
            nc.tensor.matmul(z_p[:], lhsT=sums[:], rhs=ones[:],
                             start=True, stop=True)
            invz = sb.tile([1, 1], F32)
            nc.vector.reciprocal(invz[:], z_p[:])
            izb_p = psT.tile([128, 1], F32, tag="tp", name="izb_p")
            nc.tensor.matmul(izb_p[:], lhsT=onesr[:], rhs=invz[:],
                             start=True, stop=True)
            izb = sb.tile([128, 1], F32)
            nc.vector.tensor_copy(izb[:], izb_p[:])
            attn = sb.tile([128, 16], F32)
            nc.vector.tensor_scalar_mul(attn[:], ex[:], izb[:])
            nc.sync.dma_start(out=out_at_d[:].rearrange("(p f) -> p f", f=16),
                              in_=attn[:])
            # local attention weights (own 256 scores live in sc psum)
            el = sb.tile([128, 2], F32)
            nc.scalar.activation(el[:], sc_sb[:], ACT.Exp, bias=nmb[:])
            al = sb.tile([128, 2], F32)
            nc.vector.tensor_scalar_mul(al[:], el[:], izb[:])

            # ---- P5: partial ctx over own encoder shard ------------------
            ctxp = psB.tile([128, HC], F32, tag="att")
            for j in range(HC):
                for t_i in range(2):
                    nc.tensor.matmul(
                        ctxp[:, j:j + 1],
                        lhsT=encN_sb[t_i][:, j * 128:(j + 1) * 128],
                        rhs=al[:, t_i:t_i + 1],
                        start=(t_i == 0), stop=(t_i == 1))
            ctx_sb = sb.tile([128, HC], F32)
            nc.vector.tensor_copy(ctx_sb[:], ctxp[:])
            ctx_b = dr.tile([H], F32)
            nc.sync.dma_start(out=ctx_b[:].rearrange("(p f) -> p f", f=HC),
                              in_=ctx_sb[:])
            ctx_g = drs.tile([H], F32, addr_space="Shared", name="ctx_g")
            nc.gpsimd.collective_compute("AllReduce", ALU.add,
                                         replica_groups=rg,
                                         ins=[ctx_b[:]], outs=[ctx_g[:]])
            ctx8 = sb.tile([128, HC], F32)
            nc.sync.dma_start(out=ctx8[:],
                              in_=ctx_g[:].rearrange("(p f) -> p f", f=HC))

            # ---- P6: comb slice ------------------------------------------
            cbp = psB.tile([128, 1], F32, tag="att")
            for c in range(16):
                rhs = ctx8[:, c:c + 1] if c < HC else hnew8[:, c - HC:c - HC + 1]
                nc.tensor.matmul(cbp[:], lhsT=wcb_sb[c][:], rhs=rhs,
                                 start=(c == 0), stop=(c == 15))
            cb = sb.tile([128, 1], F32)
            nc.scalar.activation(cb[:], cbp[:], ACT.Tanh, bias=bcb[:])
            cb_b = dr.tile([128], F32)
            nc.sync.dma_start(out=cb_b[:].rearrange("(p f) -> p f", f=1),
                              in_=cb[:])
            cb_g = drs.tile([H], F32, addr_space="Shared", name="cb_g")
            nc.gpsimd.collective_compute("AllGather", ALU.bypass,
                                         replica_groups=rg,
                                         ins=[cb_b[:]], outs=[cb_g[:]])
            comb8 = sb.tile([128, HC], F32)
            nc.sync.dma_start(out=comb8[:],
                              in_=cb_g[:].rearrange("(f p) -> p f", p=128))

            # ---- P7: W_out matvec + streaming stats ----------------------
            comb8b = sb.tile([128, HC], BF16)
            nc.vector.tensor_copy(comb8b[:], comb8[:])
            logits = sb.tile([128, NVT], F32)
            rmax = sb.tile([128, 1], F32)
            nc.vector.memset(rmax[:], NEG)
            for vb in range(NB):
                ps = psW.tile([128, BW // 128], F32, tag="wops",
                              name=f"wops_{vb}")
                for vt in range(BW // 128):
                    for c in range(HC):
                        nc.tensor.matmul(
                            ps[:, vt:vt + 1],
                            lhsT=wo_tiles[vb][c][:, vt * 128:(vt + 1) * 128],
                            rhs=comb8b[:, c:c + 1],
                            start=(c == 0), stop=(c == HC - 1))
                nvb = BW // 128
                nc.vector.tensor_add(logits[:, vb * nvb:(vb + 1) * nvb],
                                     ps[:], bout_sb[:, vb * nvb:(vb + 1) * nvb])
                bm = sb.tile([128, 1], F32, tag="bm", name=f"bm_{vb}")
                nc.vector.tensor_reduce(bm[:], logits[:, vb * nvb:(vb + 1) * nvb], axis=AX, op=ALU.max)
                nc.vector.tensor_max(rmax[:], rmax[:], bm[:])

            # ---- P8: log-softmax stats + final ---------------------------
            rmt = psT.tile([1, 128], F32, tag="tp", name="rmt")
            nc.tensor.transpose(rmt[:], rmax[:], ident[:])
            mk = sb.tile([1, 1], F32)
            nc.vector.tensor_reduce(mk[:], rmt[:], axis=AX, op=ALU.max)
            nmk = sb.tile([1, 1], F32)
            nc.scalar.mul(nmk[:], mk[:], -1.0)
            nmkb_p = psT.tile([128, 1], F32, tag="tp", name="nmkb_p")
            nc.tensor.matmul(nmkb_p[:], lhsT=onesr[:], rhs=nmk[:],
                             start=True, stop=True)
            nmkb = sb.tile([128, 1], F32)
            nc.vector.tensor_copy(nmkb[:], nmkb_p[:])
            esc = sb.tile([128, NVT], F32)
            sumk = sb.tile([128, 1], F32)
            nc.scalar.activation(esc[:], logits[:], ACT.Exp, bias=nmkb[:],
                                 accum_out=sumk[:])
            sk_p = psT.tile([1, 1], F32, tag="tp", name="sk_p")
            nc.tensor.matmul(sk_p[:], lhsT=sumk[:], rhs=ones[:],
                             start=True, stop=True)
            st2 = sb.tile([1, 2], F32)
            nc.vector.tensor_copy(st2[0:1, 0:1], mk[:])
            nc.vector.tensor_copy(st2[0:1, 1:2], sk_p[:])
            st_b = dr.tile([2], F32)
            nc.sync.dma_start(out=st_b[:].rearrange("(p f) -> p f", p=1),
                              in_=st2[:])
            st_g = drs.tile([2 * N_CORES], F32, addr_space="Shared",
                            name="st_g")
            nc.gpsimd.collective_compute("AllGather", ALU.bypass,
                                         replica_groups=rg,
                                         ins=[st_b[:]], outs=[st_g[:]])
            sg = sb.tile([1, 16], F32)
            nc.sync.dma_start(out=sg[:],
                              in_=st_g[:].rearrange("(p f) -> p f", p=1))
            sgv = sg[:].rearrange("p (e two) -> p two e", two=2)
            mview = sgv[:, 0:1, :].rearrange("p a e -> p (a e)")
            sview = sgv[:, 1:2, :].rearrange("p a e -> p (a e)")
            gm = sb.tile([1, 1], F32)
            nc.vector.tensor_reduce(gm[:], mview, axis=AX, op=ALU.max)
            ngm = sb.tile([1, 1], F32)
            nc.scalar.mul(ngm[:], gm[:], -1.0)
            em8 = sb.tile([1, 8], F32)
            nc.scalar.activation(em8[:], mview, ACT.Exp, bias=ngm[:])
            zt8 = sb.tile([1, 8], F32)
            nc.vector.tensor_mul(zt8[:], em8[:], sview)
            zz = sb.tile([1, 1], F32)
            nc.vector.tensor_reduce(zz[:], zt8[:], axis=AX, op=ALU.add)
            lz = sb.tile([1, 1], F32)
            nc.scalar.activation(lz[:], zz[:], ACT.Ln)
            cc = sb.tile([1, 1], F32)
            nc.vector.tensor_add(cc[:], gm[:], lz[:])
            ncc = sb.tile([1, 1], F32)
            nc.scalar.mul(ncc[:], cc[:], -1.0)
            nccb_p = psT.tile([128, 1], F32, tag="tp", name="nccb_p")
            nc.tensor.matmul(nccb_p[:], lhsT=onesr[:], rhs=ncc[:],
                             start=True, stop=True)
            nccb = sb.tile([128, 1], F32)
            nc.vector.tensor_copy(nccb[:], nccb_p[:])
            outf = sb.tile([128, NVT], F32)
            nc.vector.tensor_scalar_add(outf[:], logits[:], nccb[:])
            nc.sync.dma_start(out=out_lg_d[:], in_=outf[:])

    nc.compile()
    _NC_CACHE["nc"] = nc
    return nc


def _prep_inputs(input_ids, hidden, encoder_outputs, emb_table, W_ih, W_hh,
                 b_ih, b_hh, W_comb, b_comb, W_out, b_out):
    """Shard + pre-layout all inputs per core (host-side, numpy)."""
    f = np.float32
    x_row = np.ascontiguousarray(emb_table[int(input_ids[0])], dtype=f)
    h_row = np.ascontiguousarray(hidden.reshape(H), dtype=f)
    x8 = np.ascontiguousarray(x_row.reshape(HC, 128).T)
    h8 = np.ascontiguousarray(h_row.reshape(HC, 128).T)
    ident = np.eye(128, dtype=f)
    ones = np.ones((128, 1), dtype=f)
    onesr = np.ones((1, 128), dtype=f)
    bsum = (b_ih + b_hh).astype(f)

    encT_full = np.ascontiguousarray(encoder_outputs.T, dtype=f)  # (H, S)
    W_ih = np.asarray(W_ih, dtype=f)
    W_hh = np.asarray(W_hh, dtype=f)
    W_comb = np.asarray(W_comb, dtype=f)
    W_out = np.asarray(W_out, dtype=f)
    b_out = np.asarray(b_out, dtype=f)

    in_maps = []
    for k in range(N_CORES):
        sl = slice(k * 128, (k + 1) * 128)
        # gate-sliced, transposed GRU weights: (H, 384) cols = [r|z|n]
        wihT = np.empty((H, 384), dtype=f)
        whhT = np.empty((H, 384), dtype=f)
        for g in range(3):
            wihT[:, g * 128:(g + 1) * 128] = W_ih[g * H + k * 128:
                                                  g * H + (k + 1) * 128, :].T
            whhT[:, g * 128:(g + 1) * 128] = W_hh[g * H + k * 128:
                                                  g * H + (k + 1) * 128, :].T
        brz = np.stack([bsum[0 * H + k * 128:0 * H + (k + 1) * 128],
                        bsum[1 * H + k * 128:1 * H + (k + 1) * 128]], axis=1)
        bin_ = np.asarray(b_ih[2 * H + k * 128:2 * H + (k + 1) * 128],
                          dtype=f).reshape(128, 1)
        bhn = np.asarray(b_hh[2 * H + k * 128:2 * H + (k + 1) * 128],
                         dtype=f).reshape(128, 1)
        encT = np.ascontiguousarray(encT_full[:, k * SPC:(k + 1) * SPC])
        encN = np.ascontiguousarray(
            encoder_outputs[k * SPC:(k + 1) * SPC, :], dtype=f)
        wcbT = np.ascontiguousarray(W_comb[sl, :].T)        # (2H, 128)
        bcb = np.asarray(b_comb[sl], dtype=f).reshape(128, 1)
        lo, hi = int(_OFFS[k]), int(_OFFS[k + 1])
        r = hi - lo
        import ml_dtypes
        woutT = np.zeros((H, VPC), dtype=ml_dtypes.bfloat16)
        woutT[:, :r] = W_out[lo:hi, :].T.astype(ml_dtypes.bfloat16)
        b_pad = np.full(VPC, NEG, dtype=f)
        b_pad[:r] = b_out[lo:hi]
        bout = np.ascontiguousarray(b_pad.reshape(NVT, 128).T)  # (128, NVT)
        in_maps.append({
            "x8": x8, "h8": h8,
            "hown": np.ascontiguousarray(h_row[sl]).reshape(128, 1),
            "wihT": wihT, "whhT": whhT, "brz": np.ascontiguousarray(brz),
            "bin": bin_, "bhn": bhn, "encT": encT, "encN": encN,
            "wcbT": wcbT, "bcb": bcb, "woutT": woutT, "bout2d": bout,
            "ident": ident, "ones": ones, "onesr": onesr,
        })
    return in_maps


def _assemble(results):
    log_probs = np.empty((1, V), dtype=np.float32)
    for k in range(N_CORES):
        lg = np.asarray(results[k]["out_logits"]).reshape(128, NVT)
        shard = lg.T.reshape(VPC)
        lo, hi = int(_OFFS[k]), int(_OFFS[k + 1])
        log_probs[0, lo:hi] = shard[:hi - lo]
    h_new = np.asarray(results[0]["out_hnew"]).reshape(1, 1, H)
    attn = np.asarray(results[0]["out_attn"]).reshape(S)
    return log_probs, h_new, attn


_LAST_EXEC_NS = {"ns": None}


def kernel(**inputs):
    nc = _build_nc()
    in_maps = _prep_inputs(**inputs)
    if os.environ.get("KERNEL_SIM"):
        from concourse.bass_interp import MultiCoreSim
        sim = MultiCoreSim(nc, N_CORES)
        for i in range(N_CORES):
            for name, arr in in_maps[i].items():
                sim.cores[i].tensor(name)[:] = arr
        sim.simulate(check_with_hw=False)
        results = [{name: np.asarray(sim.cores[i].mem_tensor(name))
                    for name in ("out_logits", "out_hnew", "out_attn")}
                   for i in range(N_CORES)]
    else:
        trace = bool(os.environ.get("KERNEL_TRACE"))
        res = run_bass_kernel_spmd(nc, in_maps, list(range(N_CORES)),
                                   trace=trace)
        _LAST_EXEC_NS["ns"] = res.exec_time_ns
        results = res.results
    return _assemble(results)


# revision 11
# speedup vs baseline: 1.7198x; 1.2263x over previous
"""DecoderRNN single-step decode on 8 Trainium2 NeuronCores.

Strategy (tensor-parallel, everything sharded):
  - Host gathers the embedding row (pure indexing) and pre-transposes /
    shards all weights per core.
  - Core k computes h_new[k*128:(k+1)*128] (GRU slices), its 256 rows of
    attention scores, a partial ctx over its encoder shard, its 128-slice
    of comb, and its ~6283-row shard of W_out logits.
  - Cross-core exchanges use 5 small collectives: AllGather(h_new),
    AllGather(scores), AllReduce(ctx), AllGather(comb),
    AllGather(log-softmax stats).
  - log_softmax: per-core max m_k / sum s_k = sum exp(l - m_k); global
    C = M + log(sum_k s_k * exp(m_k - M)); each core outputs l - C.

Self-contained: shapes hardcoded, no sibling imports.
"""

import os
import numpy as np

import concourse.bacc as bacc
import concourse.bass as bass
import concourse.tile as tile
import concourse.mybir as mybir
from concourse.bass_utils import run_bass_kernel_spmd

F32 = mybir.dt.float32
BF16 = mybir.dt.bfloat16
AX = mybir.AxisListType.X
ALU = mybir.AluOpType
ACT = mybir.ActivationFunctionType

N_CORES = 8
H = 1024
V = 50257
S = 2048
HC = H // 128          # 8 h-chunks
SPC = S // N_CORES     # 256 encoder rows per core
VPC = 6400             # padded W_out rows per core (50 tiles of 128)
NVT = VPC // 128       # 50 v-tiles per core
BW = 640               # W_out DMA block width (5 v-tiles)
NB = VPC // BW         # 10 v-blocks
NEG = -1.0e9           # pad bias so padded logits never matter

_ROWS = [6283] * 7 + [V - 7 * 6283]   # real W_out rows per core
_OFFS = np.cumsum([0] + _ROWS)

_NC_CACHE = {}


def _build_nc():
    if "nc" in _NC_CACHE:
        return _NC_CACHE["nc"]
    nc = bacc.Bacc("TRN2", target_bir_lowering=False, debug=False,
                   num_devices=N_CORES)
    rg = [list(range(N_CORES))]

    # ---- per-core inputs --------------------------------------------------
    x8_d = nc.dram_tensor("x8", [128, HC], F32, kind="ExternalInput")
    h8_d = nc.dram_tensor("h8", [128, HC], F32, kind="ExternalInput")
    hown_d = nc.dram_tensor("hown", [128, 1], F32, kind="ExternalInput")
    wihT_d = nc.dram_tensor("wihT", [H, 384], F32, kind="ExternalInput")
    whhT_d = nc.dram_tensor("whhT", [H, 384], F32, kind="ExternalInput")
    brz_d = nc.dram_tensor("brz", [128, 2], F32, kind="ExternalInput")
    bin_d = nc.dram_tensor("bin", [128, 1], F32, kind="ExternalInput")
    bhn_d = nc.dram_tensor("bhn", [128, 1], F32, kind="ExternalInput")
    encT_d = nc.dram_tensor("encT", [H, SPC], F32, kind="ExternalInput")
    encN_d = nc.dram_tensor("encN", [SPC, H], F32, kind="ExternalInput")
    wcbT_d = nc.dram_tensor("wcbT", [2 * H, 128], F32, kind="ExternalInput")
    bcb_d = nc.dram_tensor("bcb", [128, 1], F32, kind="ExternalInput")
    woutT_d = nc.dram_tensor("woutT", [H, VPC], BF16, kind="ExternalInput")
    bout_d = nc.dram_tensor("bout2d", [128, NVT], F32, kind="ExternalInput")
    ident_d = nc.dram_tensor("ident", [128, 128], F32, kind="ExternalInput")
    ones_d = nc.dram_tensor("ones", [128, 1], F32, kind="ExternalInput")
    onesr_d = nc.dram_tensor("onesr", [1, 128], F32, kind="ExternalInput")

    # ---- outputs ----------------------------------------------------------
    out_lg_d = nc.dram_tensor("out_logits", [128, NVT], F32,
                              kind="ExternalOutput")
    out_hn_d = nc.dram_tensor("out_hnew", [H], F32, kind="ExternalOutput")
    out_at_d = nc.dram_tensor("out_attn", [S], F32, kind="ExternalOutput")

    with tile.TileContext(nc) as tc:
        with (
            tc.tile_pool(name="w", bufs=1) as w,          # persistent weights
            tc.tile_pool(name="wo", bufs=16) as wo,       # W_out stream
            tc.tile_pool(name="sb", bufs=1) as sb,        # small working tiles
            tc.tile_pool(name="psA", bufs=2, space="PSUM") as psA,
            tc.tile_pool(name="psB", bufs=2, space="PSUM") as psB,
            tc.tile_pool(name="psW", bufs=3, space="PSUM") as psW,
            tc.tile_pool(name="psT", bufs=1, space="PSUM") as psT,
            tc.tile_pool(name="dr", bufs=1, space="DRAM") as dr,
            tc.tile_pool(name="drs", bufs=1, space="DRAM") as drs,
        ):
            # ---- front-end weight / vector loads (priority first) --------
            ident = w.tile([128, 128], F32)
            nc.sync.dma_start(out=ident[:], in_=ident_d[:])
            ones = w.tile([128, 1], F32)
            nc.sync.dma_start(out=ones[:], in_=ones_d[:])
            onesr = w.tile([1, 128], F32)
            nc.sync.dma_start(out=onesr[:], in_=onesr_d[:])
            x8 = w.tile([128, HC], F32)
            nc.sync.dma_start(out=x8[:], in_=x8_d[:])
            h8 = w.tile([128, HC], F32)
            nc.sync.dma_start(out=h8[:], in_=h8_d[:])
            hown = w.tile([128, 1], F32)
            nc.sync.dma_start(out=hown[:], in_=hown_d[:])
            brz = w.tile([128, 2], F32)
            nc.sync.dma_start(out=brz[:], in_=brz_d[:])
            bin_ = w.tile([128, 1], F32)
            nc.sync.dma_start(out=bin_[:], in_=bin_d[:])
            bhn = w.tile([128, 1], F32)
            nc.sync.dma_start(out=bhn[:], in_=bhn_d[:])
            bcb = w.tile([128, 1], F32)
            nc.sync.dma_start(out=bcb[:], in_=bcb_d[:])

            wih_sb = []
            whh_sb = []
            for c in range(HC):
                t1 = w.tile([128, 384], F32, name=f"wih_{c}")
                nc.sync.dma_start(out=t1[:],
                                  in_=wihT_d[c * 128:(c + 1) * 128, :])
                wih_sb.append(t1)
                t2 = w.tile([128, 384], F32, name=f"whh_{c}")
                nc.sync.dma_start(out=t2[:],
                                  in_=whhT_d[c * 128:(c + 1) * 128, :])
                whh_sb.append(t2)
            encT_sb = []
            for c in range(HC):
                t = w.tile([128, SPC], F32, name=f"encT_{c}")
                nc.sync.dma_start(out=t[:],
                                  in_=encT_d[c * 128:(c + 1) * 128, :])
                encT_sb.append(t)
            encN_sb = []
            for t_i in range(2):
                t = w.tile([128, H], F32, name=f"encN_{t_i}")
                nc.sync.dma_start(out=t[:],
                                  in_=encN_d[t_i * 128:(t_i + 1) * 128, :])
                encN_sb.append(t)
            wcb_sb = []
            for c in range(16):
                t = w.tile([128, 128], F32, name=f"wcb_{c}")
                nc.sync.dma_start(out=t[:],
                                  in_=wcbT_d[c * 128:(c + 1) * 128, :])
                wcb_sb.append(t)
            bout_sb = w.tile([128, NVT], F32)
            nc.sync.dma_start(out=bout_sb[:], in_=bout_d[:])

            # ---- W_out stream DMAs (fill remaining bandwidth) -------------
            wo_tiles = [[None] * HC for _ in range(NB)]
            for vb in range(NB):
                for cp in range(HC // 2):
                    t = wo.tile([128, 2 * BW], BF16, tag="wo",
                                name=f"wo_{vb}_{cp}")
                    nc.sync.dma_start(
                        out=t[:].rearrange("p (c v) -> p c v", c=2),
                        in_=woutT_d[cp * 256:(cp + 1) * 256,
                                    vb * BW:(vb + 1) * BW].rearrange(
                                        "(c p) v -> p c v", p=128))
                    wo_tiles[vb][2 * cp] = t[:, 0:BW]
                    wo_tiles[vb][2 * cp + 1] = t[:, BW:2 * BW]

            # ---- P1: GRU gates -------------------------------------------
            xr = sb.tile([128, HC], F32)
            nc.scalar.activation(xr[:], x8[:], ACT.Relu)
            gi = psA.tile([128, 3], F32, tag="gru")
            gh = psA.tile([128, 3], F32, tag="gru")
            for g in range(3):
                for c in range(HC):
                    nc.tensor.matmul(gi[:, g:g + 1],
                                     lhsT=wih_sb[c][:, g * 128:(g + 1) * 128],
                                     rhs=xr[:, c:c + 1],
                                     start=(c == 0), stop=(c == HC - 1))
            for g in range(3):
                for c in range(HC):
                    nc.tensor.matmul(gh[:, g:g + 1],
                                     lhsT=whh_sb[c][:, g * 128:(g + 1) * 128],
                                     rhs=h8[:, c:c + 1],
                                     start=(c == 0), stop=(c == HC - 1))
            gisb = sb.tile([128, 3], F32)
            nc.vector.tensor_copy(gisb[:], gi[:])
            rzp = sb.tile([128, 2], F32)
            nc.vector.tensor_add(rzp[:], gisb[:, 0:2], gh[:, 0:2])
            nc.vector.tensor_add(rzp[:], rzp[:], brz[:])
            rz = sb.tile([128, 2], F32)
            nc.scalar.activation(rz[:], rzp[:], ACT.Sigmoid)
            npre = sb.tile([128, 1], F32)
            nc.vector.tensor_add(npre[:], gisb[:, 2:3], bin_[:])
            hnp = sb.tile([128, 1], F32)
            nc.vector.tensor_add(hnp[:], gh[:, 2:3], bhn[:])
            rhn = sb.tile([128, 1], F32)
            nc.vector.tensor_mul(rhn[:], rz[:, 0:1], hnp[:])
            nc.vector.tensor_add(npre[:], npre[:], rhn[:])
            n_t = sb.tile([128, 1], F32)
            nc.scalar.activation(n_t[:], npre[:], ACT.Tanh)
            # h_new = n + z*(h - n)
            dmn = sb.tile([128, 1], F32)
            nc.vector.tensor_sub(dmn[:], hown[:], n_t[:])
            nc.vector.tensor_mul(dmn[:], rz[:, 1:2], dmn[:])
            hno = sb.tile([128, 1], F32)
            nc.vector.tensor_add(hno[:], n_t[:], dmn[:])

            # ---- P2: AllGather h_new -------------------------------------
            hn_b = dr.tile([128], F32)
            nc.scalar.dma_start(out=hn_b[:].rearrange("(p f) -> p f", f=1),
                              in_=hno[:])
            hn_g = drs.tile([H], F32, addr_space="Shared")
            nc.gpsimd.collective_compute("AllGather", ALU.bypass,
                                         replica_groups=rg,
                                         ins=[hn_b[:]], outs=[hn_g[:]])
            nc.sync.dma_start(out=out_hn_d[:], in_=hn_g[:])
            hnew8 = sb.tile([128, HC], F32)
            nc.scalar.dma_start(out=hnew8[:],
                              in_=hn_g[:].rearrange("(f p) -> p f", p=128))

            # ---- P3: attention scores ------------------------------------
            sc = psB.tile([128, 2], F32, tag="att")
            for t_i in range(2):
                for c in range(HC):
                    nc.tensor.matmul(
                        sc[:, t_i:t_i + 1],
                        lhsT=encT_sb[c][:, t_i * 128:(t_i + 1) * 128],
                        rhs=hnew8[:, c:c + 1],
                        start=(c == 0), stop=(c == HC - 1))
            sc_sb = sb.tile([128, 2], F32)
            nc.vector.tensor_copy(sc_sb[:], sc[:])
            sc_b = dr.tile([SPC], F32)
            nc.scalar.dma_start(out=sc_b[:].rearrange("(f p) -> p f", p=128),
                              in_=sc_sb[:])
            sc_g = drs.tile([S], F32, addr_space="Shared")
            nc.gpsimd.collective_compute("AllGather", ALU.bypass,
                                         replica_groups=rg,
                                         ins=[sc_b[:]], outs=[sc_g[:]])

            # ---- P4: softmax ---------------------------------------------
            scf = sb.tile([128, 16], F32)
            nc.scalar.dma_start(out=scf[:],
                              in_=sc_g[:].rearrange("(p f) -> p f", f=16))
            m1 = sb.tile([128, 1], F32)
            nc.vector.tensor_reduce(m1[:], scf[:], axis=AX, op=ALU.max)
            m1t = psT.tile([1, 128], F32, tag="tp", name="m1t")
            nc.tensor.transpose(m1t[:], m1[:], ident[:])
            mx = sb.tile([1, 1], F32)
            nc.vector.tensor_reduce(mx[:], m1t[:], axis=AX, op=ALU.max)
            nmx = sb.tile([1, 1], F32)
            nc.scalar.mul(nmx[:], mx[:], -1.0)
            nmb_p = psT.tile([128, 1], F32, tag="tp", name="nmb_p")
            nc.tensor.matmul(nmb_p[:], lhsT=onesr[:], rhs=nmx[:],
                             start=True, stop=True)
            nmb = sb.tile([128, 1], F32)
            nc.vector.tensor_copy(nmb[:], nmb_p[:])
            ex = sb.tile([128, 16], F32)
            sums = sb.tile([128, 1], F32)
            nc.scalar.activation(ex[:], scf[:], ACT.Exp, bias=nmb[:],
                                 accum_out=sums[:])
            z_p = psT.tile([1, 1], F32, tag="tp", name="z_p")
            nc.tensor.matmul(z_p[:], lhsT=sums[:], rhs=ones[:],
                             start=True, stop=True)
            invz = sb.tile([1, 1], F32)
            nc.vector.reciprocal(invz[:], z_p[:])
            izb_p = psT.tile([128, 1], F32, tag="tp", name="izb_p")
            nc.tensor.matmul(izb_p[:], lhsT=onesr[:], rhs=invz[:],
                             start=True, stop=True)
            izb = sb.tile([128, 1], F32)
            nc.vector.tensor_copy(izb[:], izb_p[:])
            attn = sb.tile([128, 16], F32)
            nc.vector.tensor_scalar_mul(attn[:], ex[:], izb[:])
            nc.sync.dma_start(out=out_at_d[:].rearrange("(p f) -> p f", f=16),
                              in_=attn[:])
            # local attention weights (own 256 scores live in sc psum)
            el = sb.tile([128, 2], F32)
            nc.scalar.activation(el[:], sc_sb[:], ACT.Exp, bias=nmb[:])
            al = sb.tile([128, 2], F32)
            nc.vector.tensor_scalar_mul(al[:], el[:], izb[:])

            # ---- P5: partial ctx over own encoder shard ------------------
            ctxp = psB.tile([128, HC], F32, tag="att")
            for j in range(HC):
                for t_i in range(2):
                    nc.tensor.matmul(
                        ctxp[:, j:j + 1],
                        lhsT=encN_sb[t_i][:, j * 128:(j + 1) * 128],
                        rhs=al[:, t_i:t_i + 1],
                        start=(t_i == 0), stop=(t_i == 1))
            ctx_sb = sb.tile([128, HC], F32)
            nc.vector.tensor_copy(ctx_sb[:], ctxp[:])
            ctx_b = dr.tile([H], F32)
            nc.scalar.dma_start(out=ctx_b[:].rearrange("(p f) -> p f", f=HC),
                              in_=ctx_sb[:])
            ctx_g = drs.tile([H], F32, addr_space="Shared", name="ctx_g")
            nc.gpsimd.collective_compute("AllReduce", ALU.add,
                                         replica_groups=rg,
                                         ins=[ctx_b[:]], outs=[ctx_g[:]])
            ctx8 = sb.tile([128, HC], F32)
            nc.scalar.dma_start(out=ctx8[:],
                              in_=ctx_g[:].rearrange("(p f) -> p f", f=HC))

            # ---- P6: comb slice ------------------------------------------
            cbp = psB.tile([128, 1], F32, tag="att")
            for c in range(16):
                rhs = ctx8[:, c:c + 1] if c < HC else hnew8[:, c - HC:c - HC + 1]
                nc.tensor.matmul(cbp[:], lhsT=wcb_sb[c][:], rhs=rhs,
                                 start=(c == 0), stop=(c == 15))
            cb = sb.tile([128, 1], F32)
            nc.scalar.activation(cb[:], cbp[:], ACT.Tanh, bias=bcb[:])
            cb_b = dr.tile([128], F32)
            nc.scalar.dma_start(out=cb_b[:].rearrange("(p f) -> p f", f=1),
                              in_=cb[:])
            cb_g = drs.tile([H], F32, addr_space="Shared", name="cb_g")
            nc.gpsimd.collective_compute("AllGather", ALU.bypass,
                                         replica_groups=rg,
                                         ins=[cb_b[:]], outs=[cb_g[:]])
            comb8 = sb.tile([128, HC], F32)
            nc.scalar.dma_start(out=comb8[:],
                              in_=cb_g[:].rearrange("(f p) -> p f", p=128))

            # ---- P7: W_out matvec + streaming stats ----------------------
            comb8b = sb.tile([128, HC], BF16)
            nc.vector.tensor_copy(comb8b[:], comb8[:])
            logits = sb.tile([128, NVT], F32)
            rmax = sb.tile([128, 1], F32)
            nc.vector.memset(rmax[:], NEG)
            for vb in range(NB):
                ps = psW.tile([128, BW // 128], F32, tag="wops",
                              name=f"wops_{vb}")
                for vt in range(BW // 128):
                    for c in range(HC):
                        nc.tensor.matmul(
                            ps[:, vt:vt + 1],
                            lhsT=wo_tiles[vb][c][:, vt * 128:(vt + 1) * 128],
                            rhs=comb8b[:, c:c + 1],
                            start=(c == 0), stop=(c == HC - 1))
                nvb = BW // 128
                nc.vector.tensor_add(logits[:, vb * nvb:(vb + 1) * nvb],
                                     ps[:], bout_sb[:, vb * nvb:(vb + 1) * nvb])
                bm = sb.tile([128, 1], F32, tag="bm", name=f"bm_{vb}")
                nc.vector.tensor_reduce(bm[:], logits[:, vb * nvb:(vb + 1) * nvb], axis=AX, op=ALU.max)
                nc.vector.tensor_max(rmax[:], rmax[:], bm[:])

            # ---- P8: log-softmax stats + final ---------------------------
            rmt = psT.tile([1, 128], F32, tag="tp", name="rmt")
            nc.tensor.transpose(rmt[:], rmax[:], ident[:])
            mk = sb.tile([1, 1], F32)
            nc.vector.tensor_reduce(mk[:], rmt[:], axis=AX, op=ALU.max)
            nmk = sb.tile([1, 1], F32)
            nc.scalar.mul(nmk[:], mk[:], -1.0)
            nmkb_p = psT.tile([128, 1], F32, tag="tp", name="nmkb_p")
            nc.tensor.matmul(nmkb_p[:], lhsT=onesr[:], rhs=nmk[:],
                             start=True, stop=True)
            nmkb = sb.tile([128, 1], F32)
            nc.vector.tensor_copy(nmkb[:], nmkb_p[:])
            esc = sb.tile([128, NVT], F32)
            sumk = sb.tile([128, 1], F32)
            nc.scalar.activation(esc[:], logits[:], ACT.Exp, bias=nmkb[:],
                                 accum_out=sumk[:])
            sk_p = psT.tile([1, 1], F32, tag="tp", name="sk_p")
            nc.tensor.matmul(sk_p[:], lhsT=sumk[:], rhs=ones[:],
                             start=True, stop=True)
            st2 = sb.tile([1, 2], F32)
            nc.vector.tensor_copy(st2[0:1, 0:1], mk[:])
            nc.vector.tensor_copy(st2[0:1, 1:2], sk_p[:])
            st_b = dr.tile([2], F32)
            nc.scalar.dma_start(out=st_b[:].rearrange("(p f) -> p f", p=1),
                              in_=st2[:])
            st_g = drs.tile([2 * N_CORES], F32, addr_space="Shared",
                            name="st_g")
            nc.gpsimd.collective_compute("AllGather", ALU.bypass,
                                         replica_groups=rg,
                                         ins=[st_b[:]], outs=[st_g[:]])
            sg = sb.tile([1, 16], F32)
            nc.scalar.dma_start(out=sg[:],
                              in_=st_g[:].rearrange("(p f) -> p f", p=1))
            sgv = sg[:].rearrange("p (e two) -> p two e", two=2)
            mview = sgv[:, 0:1, :].rearrange("p a e -> p (a e)")
            sview = sgv[:, 1:2, :].rearrange("p a e -> p (a e)")
            gm = sb.tile([1, 1], F32)
            nc.vector.tensor_reduce(gm[:], mview, axis=AX, op=ALU.max)
            ngm = sb.tile([1, 1], F32)
            nc.scalar.mul(ngm[:], gm[:], -1.0)
            em8 = sb.tile([1, 8], F32)
            nc.scalar.activation(em8[:], mview, ACT.Exp, bias=ngm[:])
            zt8 = sb.tile([1, 8], F32)
            nc.vector.tensor_mul(zt8[:], em8[:], sview)
            zz = sb.tile([1, 1], F32)
            nc.vector.tensor_reduce(zz[:], zt8[:], axis=AX, op=ALU.add)
            lz = sb.tile([1, 1], F32)
            nc.scalar.activation(lz[:], zz[:], ACT.Ln)
            cc = sb.tile([1, 1], F32)
            nc.vector.tensor_add(cc[:], gm[:], lz[:])
            ncc = sb.tile([1, 1], F32)
            nc.scalar.mul(ncc[:], cc[:], -1.0)
            nccb_p = psT.tile([128, 1], F32, tag="tp", name="nccb_p")
            nc.tensor.matmul(nccb_p[:], lhsT=onesr[:], rhs=ncc[:],
                             start=True, stop=True)
            nccb = sb.tile([128, 1], F32)
            nc.vector.tensor_copy(nccb[:], nccb_p[:])
            outf = sb.tile([128, NVT], F32)
            nc.vector.tensor_scalar_add(outf[:], logits[:], nccb[:])
            nc.sync.dma_start(out=out_lg_d[:], in_=outf[:])

    nc.compile()
    _NC_CACHE["nc"] = nc
    return nc


def _prep_inputs(input_ids, hidden, encoder_outputs, emb_table, W_ih, W_hh,
                 b_ih, b_hh, W_comb, b_comb, W_out, b_out):
    """Shard + pre-layout all inputs per core (host-side, numpy)."""
    f = np.float32
    x_row = np.ascontiguousarray(emb_table[int(input_ids[0])], dtype=f)
    h_row = np.ascontiguousarray(hidden.reshape(H), dtype=f)
    x8 = np.ascontiguousarray(x_row.reshape(HC, 128).T)
    h8 = np.ascontiguousarray(h_row.reshape(HC, 128).T)
    ident = np.eye(128, dtype=f)
    ones = np.ones((128, 1), dtype=f)
    onesr = np.ones((1, 128), dtype=f)
    bsum = (b_ih + b_hh).astype(f)

    encT_full = np.ascontiguousarray(encoder_outputs.T, dtype=f)  # (H, S)
    W_ih = np.asarray(W_ih, dtype=f)
    W_hh = np.asarray(W_hh, dtype=f)
    W_comb = np.asarray(W_comb, dtype=f)
    W_out = np.asarray(W_out, dtype=f)
    b_out = np.asarray(b_out, dtype=f)

    in_maps = []
    for k in range(N_CORES):
        sl = slice(k * 128, (k + 1) * 128)
        # gate-sliced, transposed GRU weights: (H, 384) cols = [r|z|n]
        wihT = np.empty((H, 384), dtype=f)
        whhT = np.empty((H, 384), dtype=f)
        for g in range(3):
            wihT[:, g * 128:(g + 1) * 128] = W_ih[g * H + k * 128:
                                                  g * H + (k + 1) * 128, :].T
            whhT[:, g * 128:(g + 1) * 128] = W_hh[g * H + k * 128:
                                                  g * H + (k + 1) * 128, :].T
        brz = np.stack([bsum[0 * H + k * 128:0 * H + (k + 1) * 128],
                        bsum[1 * H + k * 128:1 * H + (k + 1) * 128]], axis=1)
        bin_ = np.asarray(b_ih[2 * H + k * 128:2 * H + (k + 1) * 128],
                          dtype=f).reshape(128, 1)
        bhn = np.asarray(b_hh[2 * H + k * 128:2 * H + (k + 1) * 128],
                         dtype=f).reshape(128, 1)
        encT = np.ascontiguousarray(encT_full[:, k * SPC:(k + 1) * SPC])
        encN = np.ascontiguousarray(
            encoder_outputs[k * SPC:(k + 1) * SPC, :], dtype=f)
        wcbT = np.ascontiguousarray(W_comb[sl, :].T)        # (2H, 128)
        bcb = np.asarray(b_comb[sl], dtype=f).reshape(128, 1)
        lo, hi = int(_OFFS[k]), int(_OFFS[k + 1])
        r = hi - lo
        import ml_dtypes
        woutT = np.zeros((H, VPC), dtype=ml_dtypes.bfloat16)
        woutT[:, :r] = W_out[lo:hi, :].T.astype(ml_dtypes.bfloat16)
        b_pad = np.full(VPC, NEG, dtype=f)
        b_pad[:r] = b_out[lo:hi]
        bout = np.ascontiguousarray(b_pad.reshape(NVT, 128).T)  # (128, NVT)
        in_maps.append({
            "x8": x8, "h8": h8,
            "hown": np.ascontiguousarray(h_row[sl]).reshape(128, 1),
            "wihT": wihT, "whhT": whhT, "brz": np.ascontiguousarray(brz),
            "bin": bin_, "bhn": bhn, "encT": encT, "encN": encN,
            "wcbT": wcbT, "bcb": bcb, "woutT": woutT, "bout2d": bout,
            "ident": ident, "ones": ones, "onesr": onesr,
        })
    return in_maps


def _assemble(results):
    log_probs = np.empty((1, V), dtype=np.float32)
    for k in range(N_CORES):
        lg = np.asarray(results[k]["out_logits"]).reshape(128, NVT)
        shard = lg.T.reshape(VPC)
        lo, hi = int(_OFFS[k]), int(_OFFS[k + 1])
        log_probs[0, lo:hi] = shard[:hi - lo]
    h_new = np.asarray(results[0]["out_hnew"]).reshape(1, 1, H)
    attn = np.asarray(results[0]["out_attn"]).reshape(S)
    return log_probs, h_new, attn


_LAST_EXEC_NS = {"ns": None}


def kernel(**inputs):
    nc = _build_nc()
    in_maps = _prep_inputs(**inputs)
    if os.environ.get("KERNEL_SIM"):
        from concourse.bass_interp import MultiCoreSim
        sim = MultiCoreSim(nc, N_CORES)
        for i in range(N_CORES):
            for name, arr in in_maps[i].items():
                sim.cores[i].tensor(name)[:] = arr
        sim.simulate(check_with_hw=False)
        results = [{name: np.asarray(sim.cores[i].mem_tensor(name))
                    for name in ("out_logits", "out_hnew", "out_attn")}
                   for i in range(N_CORES)]
    else:
        trace = bool(os.environ.get("KERNEL_TRACE"))
        res = run_bass_kernel_spmd(nc, in_maps, list(range(N_CORES)),
                                   trace=trace)
        _LAST_EXEC_NS["ns"] = res.exec_time_ns
        results = res.results
    return _assemble(results)


# revision 13
# speedup vs baseline: 1.8847x; 1.0959x over previous
"""DecoderRNN single-step decode on 8 Trainium2 NeuronCores.

Strategy (tensor-parallel, everything sharded):
  - Host gathers the embedding row (pure indexing) and pre-transposes /
    shards all weights per core.
  - Core k computes h_new[k*128:(k+1)*128] (GRU slices), its 256 rows of
    attention scores, a partial ctx over its encoder shard, its 128-slice
    of comb, and its ~6283-row shard of W_out logits.
  - Cross-core exchanges use 5 small collectives: AllGather(h_new),
    AllGather(scores), AllReduce(ctx), AllGather(comb),
    AllGather(log-softmax stats).
  - log_softmax: per-core max m_k / sum s_k = sum exp(l - m_k); global
    C = M + log(sum_k s_k * exp(m_k - M)); each core outputs l - C.

Self-contained: shapes hardcoded, no sibling imports.
"""

import os
import numpy as np

import concourse.bacc as bacc
import concourse.bass as bass
import concourse.tile as tile
import concourse.mybir as mybir
from concourse.bass_utils import run_bass_kernel_spmd

F32 = mybir.dt.float32
BF16 = mybir.dt.bfloat16
AX = mybir.AxisListType.X
ALU = mybir.AluOpType
ACT = mybir.ActivationFunctionType

N_CORES = 8
H = 1024
V = 50257
S = 2048
HC = H // 128          # 8 h-chunks
SPC = S // N_CORES     # 256 encoder rows per core
VPC = 6400             # padded W_out rows per core (50 tiles of 128)
NVT = VPC // 128       # 50 v-tiles per core
BW = 640               # W_out DMA block width (5 v-tiles)
NB = VPC // BW         # 10 v-blocks
NEG = -1.0e9           # pad bias so padded logits never matter

_ROWS = [6283] * 7 + [V - 7 * 6283]   # real W_out rows per core
_OFFS = np.cumsum([0] + _ROWS)

_NC_CACHE = {}


def _build_nc():
    if "nc" in _NC_CACHE:
        return _NC_CACHE["nc"]
    nc = bacc.Bacc("TRN2", target_bir_lowering=False, debug=False,
                   num_devices=N_CORES)
    rg = [list(range(N_CORES))]

    # ---- per-core inputs --------------------------------------------------
    x8_d = nc.dram_tensor("x8", [128, HC], F32, kind="ExternalInput")
    h8_d = nc.dram_tensor("h8", [128, HC], BF16, kind="ExternalInput")
    hown_d = nc.dram_tensor("hown", [128, 1], F32, kind="ExternalInput")
    wihT_d = nc.dram_tensor("wihT", [H, 384], BF16, kind="ExternalInput")
    whhT_d = nc.dram_tensor("whhT", [H, 384], BF16, kind="ExternalInput")
    brz_d = nc.dram_tensor("brz", [128, 2], F32, kind="ExternalInput")
    bin_d = nc.dram_tensor("bin", [128, 1], F32, kind="ExternalInput")
    bhn_d = nc.dram_tensor("bhn", [128, 1], F32, kind="ExternalInput")
    encT_d = nc.dram_tensor("encT", [H, SPC], BF16, kind="ExternalInput")
    encN_d = nc.dram_tensor("encN", [SPC, H], BF16, kind="ExternalInput")
    wcbT_d = nc.dram_tensor("wcbT", [2 * H, 128], BF16, kind="ExternalInput")
    bcb_d = nc.dram_tensor("bcb", [128, 1], F32, kind="ExternalInput")
    woutT_d = nc.dram_tensor("woutT", [H, VPC], BF16, kind="ExternalInput")
    bout_d = nc.dram_tensor("bout2d", [128, NVT], F32, kind="ExternalInput")
    ident_d = nc.dram_tensor("ident", [128, 128], F32, kind="ExternalInput")
    ones_d = nc.dram_tensor("ones", [128, 1], F32, kind="ExternalInput")
    onesr_d = nc.dram_tensor("onesr", [1, 128], F32, kind="ExternalInput")

    # ---- outputs ----------------------------------------------------------
    out_lg_d = nc.dram_tensor("out_logits", [128, NVT], F32,
                              kind="ExternalOutput")
    out_hn_d = nc.dram_tensor("out_hnew", [H], F32, kind="ExternalOutput")
    out_at_d = nc.dram_tensor("out_attn", [S], F32, kind="ExternalOutput")

    with tile.TileContext(nc) as tc:
        with (
            tc.tile_pool(name="w", bufs=1) as w,          # persistent weights
            tc.tile_pool(name="wo", bufs=16) as wo,       # W_out stream
            tc.tile_pool(name="sb", bufs=1) as sb,        # small working tiles
            tc.tile_pool(name="psA", bufs=2, space="PSUM") as psA,
            tc.tile_pool(name="psB", bufs=2, space="PSUM") as psB,
            tc.tile_pool(name="psW", bufs=3, space="PSUM") as psW,
            tc.tile_pool(name="psT", bufs=1, space="PSUM") as psT,
            tc.tile_pool(name="dr", bufs=1, space="DRAM") as dr,
            tc.tile_pool(name="drs", bufs=1, space="DRAM") as drs,
        ):
            # ---- front-end weight / vector loads (priority first) --------
            ident = w.tile([128, 128], F32)
            nc.sync.dma_start(out=ident[:], in_=ident_d[:])
            ones = w.tile([128, 1], F32)
            nc.sync.dma_start(out=ones[:], in_=ones_d[:])
            onesr = w.tile([1, 128], F32)
            nc.sync.dma_start(out=onesr[:], in_=onesr_d[:])
            x8 = w.tile([128, HC], F32)
            nc.sync.dma_start(out=x8[:], in_=x8_d[:])
            h8 = w.tile([128, HC], BF16)
            nc.sync.dma_start(out=h8[:], in_=h8_d[:])
            hown = w.tile([128, 1], F32)
            nc.sync.dma_start(out=hown[:], in_=hown_d[:])
            brz = w.tile([128, 2], F32)
            nc.sync.dma_start(out=brz[:], in_=brz_d[:])
            bin_ = w.tile([128, 1], F32)
            nc.sync.dma_start(out=bin_[:], in_=bin_d[:])
            bhn = w.tile([128, 1], F32)
            nc.sync.dma_start(out=bhn[:], in_=bhn_d[:])
            bcb = w.tile([128, 1], F32)
            nc.sync.dma_start(out=bcb[:], in_=bcb_d[:])

            wih_sb = []
            whh_sb = []
            for c in range(HC):
                t1 = w.tile([128, 384], BF16, name=f"wih_{c}")
                nc.sync.dma_start(out=t1[:],
                                  in_=wihT_d[c * 128:(c + 1) * 128, :])
                wih_sb.append(t1)
                t2 = w.tile([128, 384], BF16, name=f"whh_{c}")
                nc.sync.dma_start(out=t2[:],
                                  in_=whhT_d[c * 128:(c + 1) * 128, :])
                whh_sb.append(t2)
            encT_sb = []
            for c in range(HC):
                t = w.tile([128, SPC], BF16, name=f"encT_{c}")
                nc.sync.dma_start(out=t[:],
                                  in_=encT_d[c * 128:(c + 1) * 128, :])
                encT_sb.append(t)
            encN_sb = []
            for t_i in range(2):
                t = w.tile([128, H], BF16, name=f"encN_{t_i}")
                nc.sync.dma_start(out=t[:],
                                  in_=encN_d[t_i * 128:(t_i + 1) * 128, :])
                encN_sb.append(t)
            wcb_sb = []
            for c in range(16):
                t = w.tile([128, 128], BF16, name=f"wcb_{c}")
                nc.sync.dma_start(out=t[:],
                                  in_=wcbT_d[c * 128:(c + 1) * 128, :])
                wcb_sb.append(t)
            bout_sb = w.tile([128, NVT], F32)
            nc.sync.dma_start(out=bout_sb[:], in_=bout_d[:])

            # ---- W_out stream DMAs (fill remaining bandwidth) -------------
            wo_tiles = [[None] * HC for _ in range(NB)]
            for vb in range(NB):
                for cp in range(HC // 2):
                    t = wo.tile([128, 2 * BW], BF16, tag="wo",
                                name=f"wo_{vb}_{cp}")
                    nc.sync.dma_start(
                        out=t[:].rearrange("p (c v) -> p c v", c=2),
                        in_=woutT_d[cp * 256:(cp + 1) * 256,
                                    vb * BW:(vb + 1) * BW].rearrange(
                                        "(c p) v -> p c v", p=128))
                    wo_tiles[vb][2 * cp] = t[:, 0:BW]
                    wo_tiles[vb][2 * cp + 1] = t[:, BW:2 * BW]

            # ---- P1: GRU gates -------------------------------------------
            xr = sb.tile([128, HC], BF16)
            nc.scalar.activation(xr[:], x8[:], ACT.Relu)
            gi = psA.tile([128, 3], F32, tag="gru")
            gh = psA.tile([128, 3], F32, tag="gru")
            for g in range(3):
                for c in range(HC):
                    nc.tensor.matmul(gi[:, g:g + 1],
                                     lhsT=wih_sb[c][:, g * 128:(g + 1) * 128],
                                     rhs=xr[:, c:c + 1],
                                     start=(c == 0), stop=(c == HC - 1))
            for g in range(3):
                for c in range(HC):
                    nc.tensor.matmul(gh[:, g:g + 1],
                                     lhsT=whh_sb[c][:, g * 128:(g + 1) * 128],
                                     rhs=h8[:, c:c + 1],
                                     start=(c == 0), stop=(c == HC - 1))
            gisb = sb.tile([128, 3], F32)
            nc.vector.tensor_copy(gisb[:], gi[:])
            rzp = sb.tile([128, 2], F32)
            nc.vector.tensor_add(rzp[:], gisb[:, 0:2], gh[:, 0:2])
            nc.vector.tensor_add(rzp[:], rzp[:], brz[:])
            rz = sb.tile([128, 2], F32)
            nc.scalar.activation(rz[:], rzp[:], ACT.Sigmoid)
            npre = sb.tile([128, 1], F32)
            nc.vector.tensor_add(npre[:], gisb[:, 2:3], bin_[:])
            hnp = sb.tile([128, 1], F32)
            nc.vector.tensor_add(hnp[:], gh[:, 2:3], bhn[:])
            rhn = sb.tile([128, 1], F32)
            nc.vector.tensor_mul(rhn[:], rz[:, 0:1], hnp[:])
            nc.vector.tensor_add(npre[:], npre[:], rhn[:])
            n_t = sb.tile([128, 1], F32)
            nc.scalar.activation(n_t[:], npre[:], ACT.Tanh)
            # h_new = n + z*(h - n)
            dmn = sb.tile([128, 1], F32)
            nc.vector.tensor_sub(dmn[:], hown[:], n_t[:])
            nc.vector.tensor_mul(dmn[:], rz[:, 1:2], dmn[:])
            hno = sb.tile([128, 1], F32)
            nc.vector.tensor_add(hno[:], n_t[:], dmn[:])

            # ---- P2: AllGather h_new -------------------------------------
            hn_b = dr.tile([128], F32)
            nc.scalar.dma_start(out=hn_b[:].rearrange("(p f) -> p f", f=1),
                              in_=hno[:])
            hn_g = drs.tile([H], F32, addr_space="Shared")
            nc.gpsimd.collective_compute("AllGather", ALU.bypass,
                                         replica_groups=rg,
                                         ins=[hn_b[:]], outs=[hn_g[:]])
            nc.sync.dma_start(out=out_hn_d[:], in_=hn_g[:])
            hnew8 = sb.tile([128, HC], F32)
            nc.scalar.dma_start(out=hnew8[:],
                              in_=hn_g[:].rearrange("(f p) -> p f", p=128))
            hnew8b = sb.tile([128, HC], BF16)
            nc.vector.tensor_copy(hnew8b[:], hnew8[:])

            # ---- P3: attention scores ------------------------------------
            sc = psB.tile([128, 2], F32, tag="att")
            for t_i in range(2):
                for c in range(HC):
                    nc.tensor.matmul(
                        sc[:, t_i:t_i + 1],
                        lhsT=encT_sb[c][:, t_i * 128:(t_i + 1) * 128],
                        rhs=hnew8b[:, c:c + 1],
                        start=(c == 0), stop=(c == HC - 1))
            sc_sb = sb.tile([128, 2], F32)
            nc.vector.tensor_copy(sc_sb[:], sc[:])
            sc_b = dr.tile([SPC], F32)
            nc.scalar.dma_start(out=sc_b[:].rearrange("(f p) -> p f", p=128),
                              in_=sc_sb[:])
            sc_g = drs.tile([S], F32, addr_space="Shared")
            nc.gpsimd.collective_compute("AllGather", ALU.bypass,
                                         replica_groups=rg,
                                         ins=[sc_b[:]], outs=[sc_g[:]])

            # ---- P4: softmax ---------------------------------------------
            scf = sb.tile([128, 16], F32)
            nc.scalar.dma_start(out=scf[:],
                              in_=sc_g[:].rearrange("(p f) -> p f", f=16))
            m1 = sb.tile([128, 1], F32)
            nc.vector.tensor_reduce(m1[:], scf[:], axis=AX, op=ALU.max)
            m1t = psT.tile([1, 128], F32, tag="tp", name="m1t")
            nc.tensor.transpose(m1t[:], m1[:], ident[:])
            mx = sb.tile([1, 1], F32)
            nc.vector.tensor_reduce(mx[:], m1t[:], axis=AX, op=ALU.max)
            nmx = sb.tile([1, 1], F32)
            nc.scalar.mul(nmx[:], mx[:], -1.0)
            nmb_p = psT.tile([128, 1], F32, tag="tp", name="nmb_p")
            nc.tensor.matmul(nmb_p[:], lhsT=onesr[:], rhs=nmx[:],
                             start=True, stop=True)
            nmb = sb.tile([128, 1], F32)
            nc.vector.tensor_copy(nmb[:], nmb_p[:])
            ex = sb.tile([128, 16], F32)
            sums = sb.tile([128, 1], F32)
            nc.scalar.activation(ex[:], scf[:], ACT.Exp, bias=nmb[:],
                                 accum_out=sums[:])
            z_p = psT.tile([1, 1], F32, tag="tp", name="z_p")
            nc.tensor.matmul(z_p[:], lhsT=sums[:], rhs=ones[:],
                             start=True, stop=True)
            invz = sb.tile([1, 1], F32)
            nc.vector.reciprocal(invz[:], z_p[:])
            izb_p = psT.tile([128, 1], F32, tag="tp", name="izb_p")
            nc.tensor.matmul(izb_p[:], lhsT=onesr[:], rhs=invz[:],
                             start=True, stop=True)
            izb = sb.tile([128, 1], F32)
            nc.vector.tensor_copy(izb[:], izb_p[:])
            attn = sb.tile([128, 16], F32)
            nc.vector.tensor_scalar_mul(attn[:], ex[:], izb[:])
            nc.sync.dma_start(out=out_at_d[:].rearrange("(p f) -> p f", f=16),
                              in_=attn[:])
            # local attention weights (own 256 scores live in sc psum)
            el = sb.tile([128, 2], F32)
            nc.scalar.activation(el[:], sc_sb[:], ACT.Exp, bias=nmb[:])
            al = sb.tile([128, 2], BF16)
            nc.vector.tensor_scalar_mul(al[:], el[:], izb[:])

            # ---- P5: partial ctx over own encoder shard ------------------
            ctxp = psB.tile([128, HC], F32, tag="att")
            for j in range(HC):
                for t_i in range(2):
                    nc.tensor.matmul(
                        ctxp[:, j:j + 1],
                        lhsT=encN_sb[t_i][:, j * 128:(j + 1) * 128],
                        rhs=al[:, t_i:t_i + 1],
                        start=(t_i == 0), stop=(t_i == 1))
            ctx_sb = sb.tile([128, HC], F32)
            nc.vector.tensor_copy(ctx_sb[:], ctxp[:])
            ctx_b = dr.tile([H], F32)
            nc.scalar.dma_start(out=ctx_b[:].rearrange("(p f) -> p f", f=HC),
                              in_=ctx_sb[:])
            ctx_g = drs.tile([H], F32, addr_space="Shared", name="ctx_g")
            nc.gpsimd.collective_compute("AllReduce", ALU.add,
                                         replica_groups=rg,
                                         ins=[ctx_b[:]], outs=[ctx_g[:]])
            ctx8 = sb.tile([128, HC], F32)
            nc.scalar.dma_start(out=ctx8[:],
                              in_=ctx_g[:].rearrange("(p f) -> p f", f=HC))
            ctx8b = sb.tile([128, HC], BF16)
            nc.vector.tensor_copy(ctx8b[:], ctx8[:])

            # ---- P6: comb slice ------------------------------------------
            cbp = psB.tile([128, 1], F32, tag="att")
            for c in range(16):
                rhs = ctx8b[:, c:c + 1] if c < HC else hnew8b[:, c - HC:c - HC + 1]
                nc.tensor.matmul(cbp[:], lhsT=wcb_sb[c][:], rhs=rhs,
                                 start=(c == 0), stop=(c == 15))
            cb = sb.tile([128, 1], F32)
            nc.scalar.activation(cb[:], cbp[:], ACT.Tanh, bias=bcb[:])
            cb_b = dr.tile([128], F32)
            nc.scalar.dma_start(out=cb_b[:].rearrange("(p f) -> p f", f=1),
                              in_=cb[:])
            cb_g = drs.tile([H], F32, addr_space="Shared", name="cb_g")
            nc.gpsimd.collective_compute("AllGather", ALU.bypass,
                                         replica_groups=rg,
                                         ins=[cb_b[:]], outs=[cb_g[:]])
            comb8 = sb.tile([128, HC], F32)
            nc.scalar.dma_start(out=comb8[:],
                              in_=cb_g[:].rearrange("(f p) -> p f", p=128))

            # ---- P7: W_out matvec + streaming stats ----------------------
            comb8b = sb.tile([128, HC], BF16)
            nc.vector.tensor_copy(comb8b[:], comb8[:])
            logits = sb.tile([128, NVT], F32)
            rmax = sb.tile([128, 1], F32)
            nc.vector.memset(rmax[:], NEG)
            for vb in range(NB):
                ps = psW.tile([128, BW // 128], F32, tag="wops",
                              name=f"wops_{vb}")
                for vt in range(BW // 128):
                    for c in range(HC):
                        nc.tensor.matmul(
                            ps[:, vt:vt + 1],
                            lhsT=wo_tiles[vb][c][:, vt * 128:(vt + 1) * 128],
                            rhs=comb8b[:, c:c + 1],
                            start=(c == 0), stop=(c == HC - 1))
                nvb = BW // 128
                nc.vector.tensor_add(logits[:, vb * nvb:(vb + 1) * nvb],
                                     ps[:], bout_sb[:, vb * nvb:(vb + 1) * nvb])
                bm = sb.tile([128, 1], F32, tag="bm", name=f"bm_{vb}")
                nc.vector.tensor_reduce(bm[:], logits[:, vb * nvb:(vb + 1) * nvb], axis=AX, op=ALU.max)
                nc.vector.tensor_max(rmax[:], rmax[:], bm[:])

            # ---- P8: log-softmax stats + final ---------------------------
            rmt = psT.tile([1, 128], F32, tag="tp", name="rmt")
            nc.tensor.transpose(rmt[:], rmax[:], ident[:])
            mk = sb.tile([1, 1], F32)
            nc.vector.tensor_reduce(mk[:], rmt[:], axis=AX, op=ALU.max)
            nmk = sb.tile([1, 1], F32)
            nc.scalar.mul(nmk[:], mk[:], -1.0)
            nmkb_p = psT.tile([128, 1], F32, tag="tp", name="nmkb_p")
            nc.tensor.matmul(nmkb_p[:], lhsT=onesr[:], rhs=nmk[:],
                             start=True, stop=True)
            nmkb = sb.tile([128, 1], F32)
            nc.vector.tensor_copy(nmkb[:], nmkb_p[:])
            esc = sb.tile([128, NVT], F32)
            sumk = sb.tile([128, 1], F32)
            nc.scalar.activation(esc[:], logits[:], ACT.Exp, bias=nmkb[:],
                                 accum_out=sumk[:])
            sk_p = psT.tile([1, 1], F32, tag="tp", name="sk_p")
            nc.tensor.matmul(sk_p[:], lhsT=sumk[:], rhs=ones[:],
                             start=True, stop=True)
            st2 = sb.tile([1, 2], F32)
            nc.vector.tensor_copy(st2[0:1, 0:1], mk[:])
            nc.vector.tensor_copy(st2[0:1, 1:2], sk_p[:])
            st_b = dr.tile([2], F32)
            nc.scalar.dma_start(out=st_b[:].rearrange("(p f) -> p f", p=1),
                              in_=st2[:])
            st_g = drs.tile([2 * N_CORES], F32, addr_space="Shared",
                            name="st_g")
            nc.gpsimd.collective_compute("AllGather", ALU.bypass,
                                         replica_groups=rg,
                                         ins=[st_b[:]], outs=[st_g[:]])
            sg = sb.tile([1, 16], F32)
            nc.scalar.dma_start(out=sg[:],
                              in_=st_g[:].rearrange("(p f) -> p f", p=1))
            sgv = sg[:].rearrange("p (e two) -> p two e", two=2)
            mview = sgv[:, 0:1, :].rearrange("p a e -> p (a e)")
            sview = sgv[:, 1:2, :].rearrange("p a e -> p (a e)")
            gm = sb.tile([1, 1], F32)
            nc.vector.tensor_reduce(gm[:], mview, axis=AX, op=ALU.max)
            ngm = sb.tile([1, 1], F32)
            nc.scalar.mul(ngm[:], gm[:], -1.0)
            em8 = sb.tile([1, 8], F32)
            nc.scalar.activation(em8[:], mview, ACT.Exp, bias=ngm[:])
            zt8 = sb.tile([1, 8], F32)
            nc.vector.tensor_mul(zt8[:], em8[:], sview)
            zz = sb.tile([1, 1], F32)
            nc.vector.tensor_reduce(zz[:], zt8[:], axis=AX, op=ALU.add)
            lz = sb.tile([1, 1], F32)
            nc.scalar.activation(lz[:], zz[:], ACT.Ln)
            cc = sb.tile([1, 1], F32)
            nc.vector.tensor_add(cc[:], gm[:], lz[:])
            ncc = sb.tile([1, 1], F32)
            nc.scalar.mul(ncc[:], cc[:], -1.0)
            nccb_p = psT.tile([128, 1], F32, tag="tp", name="nccb_p")
            nc.tensor.matmul(nccb_p[:], lhsT=onesr[:], rhs=ncc[:],
                             start=True, stop=True)
            nccb = sb.tile([128, 1], F32)
            nc.vector.tensor_copy(nccb[:], nccb_p[:])
            outf = sb.tile([128, NVT], F32)
            nc.vector.tensor_scalar_add(outf[:], logits[:], nccb[:])
            nc.sync.dma_start(out=out_lg_d[:], in_=outf[:])

    nc.compile()
    _NC_CACHE["nc"] = nc
    return nc


def _prep_inputs(input_ids, hidden, encoder_outputs, emb_table, W_ih, W_hh,
                 b_ih, b_hh, W_comb, b_comb, W_out, b_out):
    """Shard + pre-layout all inputs per core (host-side, numpy)."""
    f = np.float32
    x_row = np.ascontiguousarray(emb_table[int(input_ids[0])], dtype=f)
    h_row = np.ascontiguousarray(hidden.reshape(H), dtype=f)
    import ml_dtypes
    bf = ml_dtypes.bfloat16
    x8 = np.ascontiguousarray(x_row.reshape(HC, 128).T)
    h8 = np.ascontiguousarray(h_row.reshape(HC, 128).T.astype(bf))
    ident = np.eye(128, dtype=f)
    ones = np.ones((128, 1), dtype=f)
    onesr = np.ones((1, 128), dtype=f)
    bsum = (b_ih + b_hh).astype(f)

    encT_full = np.ascontiguousarray(encoder_outputs.T, dtype=f)  # (H, S)
    W_ih = np.asarray(W_ih, dtype=f)
    W_hh = np.asarray(W_hh, dtype=f)
    W_comb = np.asarray(W_comb, dtype=f)
    W_out = np.asarray(W_out, dtype=f)
    b_out = np.asarray(b_out, dtype=f)

    in_maps = []
    for k in range(N_CORES):
        sl = slice(k * 128, (k + 1) * 128)
        # gate-sliced, transposed GRU weights: (H, 384) cols = [r|z|n]
        wihT = np.empty((H, 384), dtype=bf)
        whhT = np.empty((H, 384), dtype=bf)
        for g in range(3):
            wihT[:, g * 128:(g + 1) * 128] = W_ih[g * H + k * 128:
                                                  g * H + (k + 1) * 128, :].T
            whhT[:, g * 128:(g + 1) * 128] = W_hh[g * H + k * 128:
                                                  g * H + (k + 1) * 128, :].T
        brz = np.stack([bsum[0 * H + k * 128:0 * H + (k + 1) * 128],
                        bsum[1 * H + k * 128:1 * H + (k + 1) * 128]], axis=1)
        bin_ = np.asarray(b_ih[2 * H + k * 128:2 * H + (k + 1) * 128],
                          dtype=f).reshape(128, 1)
        bhn = np.asarray(b_hh[2 * H + k * 128:2 * H + (k + 1) * 128],
                         dtype=f).reshape(128, 1)
        encT = np.ascontiguousarray(encT_full[:, k * SPC:(k + 1) * SPC].astype(bf))
        encN = np.ascontiguousarray(
            np.asarray(encoder_outputs, dtype=f)[k * SPC:(k + 1) * SPC, :].astype(bf))
        wcbT = np.ascontiguousarray(W_comb[sl, :].T.astype(bf))  # (2H, 128)
        bcb = np.asarray(b_comb[sl], dtype=f).reshape(128, 1)
        lo, hi = int(_OFFS[k]), int(_OFFS[k + 1])
        r = hi - lo
        woutT = np.zeros((H, VPC), dtype=bf)
        woutT[:, :r] = W_out[lo:hi, :].T.astype(bf)
        b_pad = np.full(VPC, NEG, dtype=f)
        b_pad[:r] = b_out[lo:hi]
        bout = np.ascontiguousarray(b_pad.reshape(NVT, 128).T)  # (128, NVT)
        in_maps.append({
            "x8": x8, "h8": h8,
            "hown": np.ascontiguousarray(h_row[sl]).reshape(128, 1),
            "wihT": wihT, "whhT": whhT, "brz": np.ascontiguousarray(brz),
            "bin": bin_, "bhn": bhn, "encT": encT, "encN": encN,
            "wcbT": wcbT, "bcb": bcb, "woutT": woutT, "bout2d": bout,
            "ident": ident, "ones": ones, "onesr": onesr,
        })
    return in_maps


def _assemble(results):
    log_probs = np.empty((1, V), dtype=np.float32)
    for k in range(N_CORES):
        lg = np.asarray(results[k]["out_logits"]).reshape(128, NVT)
        shard = lg.T.reshape(VPC)
        lo, hi = int(_OFFS[k]), int(_OFFS[k + 1])
        log_probs[0, lo:hi] = shard[:hi - lo]
    h_new = np.asarray(results[0]["out_hnew"]).reshape(1, 1, H)
    attn = np.asarray(results[0]["out_attn"]).reshape(S)
    return log_probs, h_new, attn


_LAST_EXEC_NS = {"ns": None}


def kernel(**inputs):
    nc = _build_nc()
    in_maps = _prep_inputs(**inputs)
    if os.environ.get("KERNEL_SIM"):
        from concourse.bass_interp import MultiCoreSim
        sim = MultiCoreSim(nc, N_CORES)
        for i in range(N_CORES):
            for name, arr in in_maps[i].items():
                sim.cores[i].tensor(name)[:] = arr
        sim.simulate(check_with_hw=False)
        results = [{name: np.asarray(sim.cores[i].mem_tensor(name))
                    for name in ("out_logits", "out_hnew", "out_attn")}
                   for i in range(N_CORES)]
    else:
        trace = bool(os.environ.get("KERNEL_TRACE"))
        res = run_bass_kernel_spmd(nc, in_maps, list(range(N_CORES)),
                                   trace=trace)
        _LAST_EXEC_NS["ns"] = res.exec_time_ns
        results = res.results
    return _assemble(results)
